# revision 1
# baseline (speedup 1.0000x reference)
"""Trainium2 Bass kernel for nn_BetweenClusterFC.

Computes out[e] = (emb_1[f[e]] @ W1 + b1) . (emb_2[t[e]] @ W2 + b2)
for E = 1.6M edges over N = 100k nodes, D_IN = 256, D_OUT = 128.

Strategy (8 NeuronCores, SPMD, full inputs in / full output out):
  - Nodes are split into 8 blocks of 12500.  Edges are assigned to cores by a
    (from-block-group, to-block-group) 4x2 rectangle: core c=(a,b) handles
    edges with from-node in blocks [4a..4a+3] and to-node in blocks
    [2b..2b+1].  Uniform (~200k edges/core), and each core only needs
    projections for 4 from-blocks + 2 to-blocks (75k nodes) instead of a
    fully replicated 200k -> far less HBM traffic.
  - Each core projects its 6 blocks (p = emb @ W + b) on the PE from
    host-pre-transposed embedding shards, writing p1/p2 tables to local DRAM.
  - Edges are bucketed host-side by (local from-block, local to-block) into
    8 buckets/core; per bucket both endpoint rows are fetched with the SWDGE
    dma_gather instruction (int16 local indices, 512B rows), then a DVE
    multiply + reduce produces the per-edge dot products.
  - The host applies the inverse edge permutation to assemble the output.

Written in raw Bass (explicit semaphores) — the Tile layer's generated sync
exceeds this toolchain's per-instruction wait-slot limits.
"""

import contextlib
import math

import numpy as np

import concourse.bass as bass
import concourse.mybir as mybir

# ---------------------------------------------------------------- constants
N_NODES = 100_000
D_IN = 256
D_OUT = 128
N_EDGES = 1_600_000
N_CORES = 8

NB = 12_500          # nodes per block
NBP = 12_544         # padded block rows (98 * 128)
NFB = 4              # from-blocks per core
NTB = 2              # to-blocks per core
NBUCKET = NFB * NTB  # 8 buckets per core

CAP = 26_624         # padded edge capacity per bucket (mean 25k, +10 sigma)
CALLS = [1024] * 26          # dma_gather call sizes (HW limit: <=1024 idxs/call)
assert sum(CALLS) == CAP
CALL_COLS = [g // 16 for g in CALLS]   # idx columns per call (wrapped by 16)
CALL_SLOTS = [g // 128 for g in CALLS]  # result slots per call
SLOT_TOT = CAP // 128                  # 208 result columns per bucket
COLS_PER_BUCKET = CAP // 16            # 1664 idx columns per bucket
IDX_COLS = NBUCKET * COLS_PER_BUCKET   # 13312

P1_ROWS = NFB * NBP  # 50176
P2_ROWS = NTB * NBP  # 25088

TILES1 = P1_ROWS // 128    # 392 node-tiles, table 1
TILES2 = P2_ROWS // 128    # 196 node-tiles, table 2
GROUPS1 = TILES1 // 4      # 98 psum groups
GROUPS2 = TILES2 // 4      # 49
NGROUP = GROUPS1 + GROUPS2  # 147
CHUNK_T = 14               # node-tiles per embT load chunk
NCH1 = TILES1 // CHUNK_T   # 28 chunks
NCH2 = TILES2 // CHUNK_T   # 14
NCHUNK = NCH1 + NCH2       # 42
EMB_COLS = CHUNK_T * 128   # 1792

NCALL = NBUCKET * len(CALLS)  # 56 gather calls per side

F32 = mybir.dt.float32
I16 = mybir.dt.int16
AX = mybir.AxisListType


# Processing order: p2 groups first, then p1 -> p-blocks finish progressively
# (p2b0@25, p2b1@49, p1b0@74, p1b1@98, p1b2@123, p1b3@147 positions), letting
# fi-major gather buckets start while later p1 blocks still project.
GSEQ = list(range(GROUPS1, NGROUP)) + list(range(GROUPS1))
CSEQ = list(range(NCH1, NCHUNK)) + list(range(NCH1))
CPOS = {cid: q for q, cid in enumerate(CSEQ)}
# pool gate positions: bucket group fi ready after this many processed groups
FI_READY = [49 + math.ceil(24.5 * (fi + 1)) for fi in range(NFB)]  # 74,98,123,147
INTERLEAVE_Q = 76  # start draining gather calls into the DVE stream here


def _chunk_of_tile(tg):
    """global tile index -> (global chunk id, table, local col0)."""
    if tg < TILES1:
        c = tg // CHUNK_T
        return c, 0, (tg % CHUNK_T) * 128
    t2 = tg - TILES1
    c = NCH1 + t2 // CHUNK_T
    return c, 1, (t2 % CHUNK_T) * 128


def _chunk_last_tile(c):
    """global chunk id -> global index of its last tile."""
    if c < NCH1:
        return (c + 1) * CHUNK_T - 1
    return TILES1 + (c - NCH1 + 1) * CHUNK_T - 1


def _chunk_src(c):
    """global chunk id -> (table, col0)."""
    if c < NCH1:
        return 0, c * EMB_COLS
    return 1, (c - NCH1) * EMB_COLS


# ---------------------------------------------------------------- device code
def build_bass(phase="all"):
    """phase: "all" | "proj" (p tables as outputs, no gather) |
    "gather" (p tables as inputs, no projection).  Non-"all" modes exist for
    hardware bring-up/debugging."""
    nc = bass.Bass()

    e1t = nc.dram_tensor("e1t", [D_IN, P1_ROWS], F32, kind="ExternalInput")
    e2t = nc.dram_tensor("e2t", [D_IN, P2_ROWS], F32, kind="ExternalInput")
    w1 = nc.dram_tensor("w1", [D_IN, D_OUT], F32, kind="ExternalInput")
    w2 = nc.dram_tensor("w2", [D_IN, D_OUT], F32, kind="ExternalInput")
    b1f = nc.dram_tensor("b1f", [128, 512], F32, kind="ExternalInput")
    b2f = nc.dram_tensor("b2f", [128, 512], F32, kind="ExternalInput")
    idxa = nc.dram_tensor("idxa", [128, IDX_COLS], I16, kind="ExternalInput")
    idxb = nc.dram_tensor("idxb", [128, IDX_COLS], I16, kind="ExternalInput")
    res = nc.dram_tensor("res", [NBUCKET, 128, SLOT_TOT], F32, kind="ExternalOutput")

    pkind = {"all": "Internal", "proj": "ExternalOutput", "gather": "ExternalInput"}[phase]
    p1d = nc.dram_tensor("p1d", [P1_ROWS, D_OUT], F32, kind=pkind)
    p2d = nc.dram_tensor("p2d", [P2_ROWS, D_OUT], F32, kind=pkind)
    pdst = (p1d, p2d)
    do_proj = phase in ("all", "proj")
    do_gather = phase in ("all", "gather")

    st = contextlib.ExitStack()
    with st:
        sb = lambda nm, shape, dt=F32: st.enter_context(nc.sbuf_tensor(nm, shape, dt))
        sem = lambda nm: st.enter_context(nc.semaphore(name=nm))

        w1c = sb("w1c", [128, 256])
        w2c = sb("w2c", [128, 256])
        bt = (sb("bt1", [128, 512]), sb("bt2", [128, 512]))
        idxt = (sb("idxta", [128, IDX_COLS], I16), sb("idxtb", [128, IDX_COLS], I16))
        et = [[sb(f"et_{p}_{k}", [128, EMB_COLS]) for k in range(2)]
              for p in range(2)]  # [parity][k]
        pv = [sb(f"pv{i}", [128, 512]) for i in range(4)]
        ps = [st.enter_context(nc.psum_tensor(f"ps{i}", [128, 512], F32))
              for i in range(4)]
        at = [sb(f"at{i}", [128, 8 * 128]) for i in range(4)]
        btg = [sb(f"btg{i}", [128, 8 * 128]) for i in range(4)]
        rt = [sb(f"rt{i}", [128, SLOT_TOT]) for i in range(4)]

        s_cl = sem("s_cl")               # const loads (8 dmas -> 128)
        s_load = (sem("s_load0"), sem("s_load1"))  # embT loads, by chunk parity
        s_mm = sem("s_mm")               # matmuls (+1 each; 2 per tile)
        s_bias = sem("s_bias")           # bias adds (+1 per group)
        s_pw = tuple(sem(f"s_pw{i}") for i in range(4))  # p-write dmas, by g%4
        s_g = tuple(sem(f"s_g{i}") for i in range(4))  # gathers, by k%4 (+16, 32/call)
        s_mul = sem("s_mul")             # muls (+1 per call)
        s_red = sem("s_red")             # reduces (+1 per call)
        s_out = tuple(sem(f"s_out{i}") for i in range(4))  # res dmas, by bk%4

        CONSTS = 8 * 16  # 8 const dmas

        block = st.enter_context(nc.Block())

        # ------------------------------------------------ SP: all HWDGE DMAs
        def _sp_proj(load_chunk, sync):
            load_chunk(0)
            load_chunk(1)
            next_cq = 2
            for q, g in enumerate(GSEQ):
                # look ahead: issue loads for chunks starting within 3 groups
                while next_cq < NCHUNK and next_cq * CHUNK_T <= (q + 3) * 4 + 3:
                    load_chunk(next_cq)
                    next_cq += 1
                sync.wait_ge(s_bias, q + 1)
                tab = 0 if g < GROUPS1 else 1
                r0 = g * 512 if tab == 0 else (g - GROUPS1) * 512
                sync.dma_start(
                    out=pdst[tab][r0:r0 + 512, :].rearrange("(t p) d -> p t d", p=128),
                    in_=pv[q % 4][:].rearrange("p (t d) -> p t d", d=128),
                ).then_inc(s_pw[q % 4], 16)
            if not do_gather:
                for r in range(4):
                    sync.wait_ge(s_pw[r], 16 * len(range(r, NGROUP, 4)))

        @block.sync
        def _(sync):
            for k in range(2):
                sync.dma_start(out=w1c[:, k * 128:(k + 1) * 128],
                               in_=w1[k * 128:(k + 1) * 128, :]).then_inc(s_cl, 16)
                sync.dma_start(out=w2c[:, k * 128:(k + 1) * 128],
                               in_=w2[k * 128:(k + 1) * 128, :]).then_inc(s_cl, 16)
            sync.dma_start(out=bt[0][:], in_=b1f[:]).then_inc(s_cl, 16)
            sync.dma_start(out=bt[1][:], in_=b2f[:]).then_inc(s_cl, 16)
            sync.dma_start(out=idxt[0][:], in_=idxa[:]).then_inc(s_cl, 16)
            sync.dma_start(out=idxt[1][:], in_=idxb[:]).then_inc(s_cl, 16)

            def load_chunk(cq):
                if cq >= 2:
                    # buffer cq%2 previously held chunk cq-2; wait until consumed
                    sync.wait_ge(s_mm, 2 * CHUNK_T * (cq - 1))
                tab, col0 = _chunk_src(CSEQ[cq])
                src = e1t if tab == 0 else e2t
                par = cq % 2
                sync.dma_start(out=et[par][0][:],
                               in_=src[0:128, col0:col0 + EMB_COLS]).then_inc(s_load[par], 16)
                sync.dma_start(out=et[par][1][:],
                               in_=src[128:256, col0:col0 + EMB_COLS]).then_inc(s_load[par], 16)

            if do_proj:
                _sp_proj(load_chunk, sync)

            if not do_gather:
                return
            for bk in range(NBUCKET):
                sync.wait_ge(s_red, len(CALLS) * (bk + 1))
                sync.dma_start(out=res[bk], in_=rt[bk % 4][:]).then_inc(s_out[bk % 4], 16)
            for r in range(4):
                sync.wait_ge(s_out[r], 16 * len(range(r, NBUCKET, 4)))

        # ------------------------------------------------ PE: projections
        @block.tensor
        def _(tensor):
            if not do_proj:
                return
            tensor.wait_ge(s_cl, CONSTS)
            for q, g in enumerate(GSEQ):
                tab = 0 if g < GROUPS1 else 1
                wc = w1c if tab == 0 else w2c
                for j in range(4):
                    tq = q * 4 + j
                    cid, _, col0 = _chunk_of_tile(g * 4 + j)
                    cq = CPOS[cid]
                    if tq == cq * CHUNK_T:  # first processed tile of chunk
                        tensor.wait_ge(s_load[cq % 2], 32 * (cq // 2 + 1))
                    if j == 0 and q >= 4:
                        tensor.wait_ge(s_bias, q - 3)  # psum bank q%4 free
                    out = ps[q % 4][:, j * 128:(j + 1) * 128]
                    tensor.matmul(out=out, lhsT=et[cq % 2][0][:, col0:col0 + 128],
                                  rhs=wc[:, 0:128], start=True, stop=False).then_inc(s_mm, 1)
                    tensor.matmul(out=out, lhsT=et[cq % 2][1][:, col0:col0 + 128],
                                  rhs=wc[:, 128:256], start=False, stop=True).then_inc(s_mm, 1)

        # ------------------------------------------------ DVE: bias + dot
        @block.vector
        def _(vector):
            def emit_call(k):
                bk, ci = k // len(CALLS), k % len(CALLS)
                S = CALL_SLOTS[ci]
                scol = sum(CALL_SLOTS[:ci])
                vector.wait_ge(s_g[k % 4], 32 * (k // 4 + 1))
                if ci == 0 and bk >= 4:
                    vector.wait_ge(s_out[bk % 4], 16 * (bk // 4))  # rt[bk%4] drained
                a3 = at[k % 4][:, :S * 128]
                b3 = btg[k % 4][:, :S * 128]
                vector.tensor_mul(out=a3, in0=a3, in1=b3).then_inc(s_mul, 1)
                vector.wait_ge(s_mul, k + 1)
                vector.reduce_sum(
                    out=rt[bk % 4][:, scol:scol + S],
                    in_=at[k % 4][:, :S * 128].rearrange("p (s d) -> p s d", d=128),
                    axis=AX.X,
                ).then_inc(s_red, 1)

            vector.wait_ge(s_cl, CONSTS)
            next_k = 0
            for q, g in enumerate(GSEQ) if do_proj else ():
                vector.wait_ge(s_mm, 8 * q + 8)
                if q >= 4:
                    vector.wait_ge(s_pw[q % 4], 16 * (q // 4))  # pv[q%4] drained
                tab = 0 if g < GROUPS1 else 1
                vector.tensor_add(out=pv[q % 4][:], in0=ps[q % 4][:],
                                  in1=bt[tab][:]).then_inc(s_bias, 1)
                if do_gather and q >= INTERLEAVE_Q and next_k < NCALL:
                    emit_call(next_k)
                    next_k += 1
            while do_gather and next_k < NCALL:
                emit_call(next_k)
                next_k += 1

        # ------------------------------------------------ Pool: gathers
        @block.gpsimd
        def _(gpsimd):
            if not do_gather:
                return
            from concourse import library_config
            gpsimd.load_library(library_config.mlp)
            regs = {gsz: gpsimd.to_reg(gsz) for gsz in sorted(set(CALLS))}
            gpsimd.wait_ge(s_cl, CONSTS)
            gated_fi = -1
            for k in range(NCALL):
                bk, ci = k // len(CALLS), k % len(CALLS)
                if do_proj and ci == 0 and bk // NTB > gated_fi:
                    gated_fi = bk // NTB
                    n = FI_READY[gated_fi]
                    for r in range(4):
                        gpsimd.wait_ge(s_pw[r], 16 * len(range(r, n, 4)))
                fi, ti = bk // NTB, bk % NTB
                gsz = CALLS[ci]
                S = CALL_SLOTS[ci]
                col0 = bk * COLS_PER_BUCKET + sum(CALL_COLS[:ci])
                ncols = CALL_COLS[ci]
                if k >= 4:
                    gpsimd.wait_ge(s_red, k - 3)  # at/bt[k%4] consumed
                gpsimd.dma_gather(
                    out_ap=at[k % 4][:, :S * 128].rearrange("p (s d) -> p s d", d=128),
                    in_ap=p1d[fi * NBP:(fi + 1) * NBP, :],
                    idxs_ap=idxt[0][:, col0:col0 + ncols],
                    num_idxs=gsz, num_idxs_reg=regs[gsz], elem_size=D_OUT,
                    queue_num=0,
                ).then_inc(s_g[k % 4], 16)
                gpsimd.dma_gather(
                    out_ap=btg[k % 4][:, :S * 128].rearrange("p (s d) -> p s d", d=128),
                    in_ap=p2d[ti * NBP:(ti + 1) * NBP, :],
                    idxs_ap=idxt[1][:, col0:col0 + ncols],
                    num_idxs=gsz, num_idxs_reg=regs[gsz], elem_size=D_OUT,
                    queue_num=0,
                ).then_inc(s_g[k % 4], 16)

    return nc


_NC_CACHE = None


def _get_nc():
    global _NC_CACHE
    if _NC_CACHE is None:
        nc = build_bass()
        from concourse.library_overlay import lower_extended_insts
        lower_extended_insts(nc)
        _NC_CACHE = nc
    return _NC_CACHE


# ---------------------------------------------------------------- host side
def _marshal(emb_1, emb_2, nodes_from_to, W1, b1, W2, b2):
    """Shard/bucket inputs per core.  Returns (in_maps, bookkeeping)."""
    f = np.asarray(nodes_from_to[:, 0], dtype=np.int64)
    t = np.asarray(nodes_from_to[:, 1], dtype=np.int64)
    emb_1 = np.ascontiguousarray(np.asarray(emb_1, dtype=np.float32))
    emb_2 = np.ascontiguousarray(np.asarray(emb_2, dtype=np.float32))
    W1 = np.asarray(W1, dtype=np.float32)
    W2 = np.asarray(W2, dtype=np.float32)
    b1 = np.asarray(b1, dtype=np.float32).reshape(-1)
    b2 = np.asarray(b2, dtype=np.float32).reshape(-1)

    core = (f // (NFB * NB)) * 4 + t // (NTB * NB)
    order0 = np.argsort(core, kind="stable")
    ccnt = np.bincount(core, minlength=N_CORES)
    coff = np.concatenate([[0], np.cumsum(ccnt)])

    b1f = np.tile(b1.reshape(1, D_OUT), (128, 4)).astype(np.float32)
    b2f = np.tile(b2.reshape(1, D_OUT), (128, 4)).astype(np.float32)

    in_maps, books = [], []
    for c in range(N_CORES):
        a, b = c // 4, c % 4
        sel = order0[coff[c]:coff[c + 1]]
        fc, tcv = f[sel], t[sel]
        fi = fc // NB - NFB * a
        ti = tcv // NB - NTB * b
        fl = (fc % NB).astype(np.int16)
        tl = (tcv % NB).astype(np.int16)
        bk = fi * NTB + ti
        o2 = np.argsort(bk, kind="stable")
        sel2, fl2, tl2 = sel[o2], fl[o2], tl[o2]
        cnts = np.bincount(bk, minlength=NBUCKET)
        if (cnts > CAP).any():
            raise RuntimeError(f"bucket overflow on core {c}: {cnts}")
        pos = np.concatenate([[0], np.cumsum(cnts)])

        slots_a = np.zeros((NBUCKET, CAP), np.int16)
        slots_b = np.zeros((NBUCKET, CAP), np.int16)
        for k in range(NBUCKET):
            slots_a[k, :cnts[k]] = fl2[pos[k]:pos[k + 1]]
            slots_b[k, :cnts[k]] = tl2[pos[k]:pos[k + 1]]
        # wrap by 16: idx i of a bucket at (partition i%16, col i//16),
        # replicated across the 8 groups of 16 partitions
        wa = slots_a.reshape(NBUCKET, CAP // 16, 16).transpose(0, 2, 1)
        wb = slots_b.reshape(NBUCKET, CAP // 16, 16).transpose(0, 2, 1)
        idxa = np.concatenate([np.tile(wa[k], (8, 1)) for k in range(NBUCKET)], axis=1)
        idxb = np.concatenate([np.tile(wb[k], (8, 1)) for k in range(NBUCKET)], axis=1)

        e1t = np.zeros((D_IN, P1_ROWS), np.float32)
        for i in range(NFB):
            blk = emb_1[(NFB * a + i) * NB:(NFB * a + i + 1) * NB]
            e1t[:, i * NBP:i * NBP + NB] = blk.T
        e2t = np.zeros((D_IN, P2_ROWS), np.float32)
        for i in range(NTB):
            blk = emb_2[(NTB * b + i) * NB:(NTB * b + i + 1) * NB]
            e2t[:, i * NBP:i * NBP + NB] = blk.T

        in_maps.append({
            "e1t": e1t, "e2t": e2t,
            "w1": W1, "w2": W2, "b1f": b1f, "b2f": b2f,
            "idxa": np.ascontiguousarray(idxa),
            "idxb": np.ascontiguousarray(idxb),
        })
        books.append((sel2, cnts, pos))
    return in_maps, books


def _unmarshal(results, books, n_edges):
    out = np.empty(n_edges, np.float32)
    scol0 = np.concatenate([[0], np.cumsum(CALL_SLOTS)])
    for c in range(N_CORES):
        sel2, cnts, pos = books[c]
        r = results[c]["res"]  # [NBUCKET, 128, SLOT_TOT]
        for k in range(NBUCKET):
            if cnts[k] == 0:
                continue
            arr = r[k]
            stream = np.concatenate([
                arr[:, scol0[ci]:scol0[ci] + CALL_SLOTS[ci]].T.reshape(-1)
                for ci in range(len(CALLS))
            ])
            out[sel2[pos[k]:pos[k + 1]]] = stream[:cnts[k]]
    return out


def _run(inputs, trace=False, **run_kwargs):
    from concourse.bass_utils import run_bass_kernel_spmd

    nc = _get_nc()
    in_maps, books = _marshal(**inputs)
    r = run_bass_kernel_spmd(
        nc, in_maps, core_ids=list(range(N_CORES)), trace=trace, **run_kwargs
    )
    out = _unmarshal(r.results, books, len(inputs["nodes_from_to"]))
    return out, r


def kernel(**inputs) -> np.ndarray:
    out, _ = _run(inputs, trace=False)
    return out



# revision 19
# speedup vs baseline: 2.0024x; 2.0024x over previous
"""Trainium2 Bass kernel for nn_BetweenClusterFC.

Computes out[e] = (emb_1[f[e]] @ W1 + b1) . (emb_2[t[e]] @ W2 + b2)
for E = 1.6M edges over N = 100k nodes, D_IN = 256, D_OUT = 128.

Strategy (8 NeuronCores, SPMD, full inputs in / full output out):
  - Nodes are split into 8 blocks of 12500.  Edges are assigned to cores by a
    (from-block-group, to-block-group) 4x2 rectangle: core c=(a,b) handles
    edges with from-node in blocks [4a..4a+3] and to-node in blocks
    [2b..2b+1].  Uniform (~200k edges/core); each core needs projections for
    4 from-blocks + 2 to-blocks (75k nodes).
  - All streamed data is bf16: the host pre-transposes and casts the
    embedding shards; the PE projects p = emb @ W (+ bias folded in as a
    K=1 ones x bias matmul), the Act engine moves psum -> sbuf in bf16 and
    issues the p-table writes (1KB-contiguous via a (p t)-permuted row
    order), keeping the DVE entirely free for the edge phase.
  - Edges are bucketed host-side by (local from-block, local to-block); per
    bucket both endpoint rows are fetched with SWDGE dma_gather calls (up to
    2048 idxs/call, two queues) using int16 permuted-local indices; a DVE
    bf16 multiply (2x mode) + f32 reduce produces the per-edge dots into a
    single SBUF-resident result tile, written out once at the end.
  - Bucket capacities are sized from the actual per-core edge counts (max
    across cores, 128-aligned), so the Bass program is built per problem
    instance (inspector-executor style) and cached.
  - The host applies the inverse edge permutation to assemble the output.

Written in raw Bass (explicit semaphores) - the Tile layer's generated sync
exceeds this toolchain's per-instruction wait-slot limits.
"""

import contextlib

import numpy as np

import concourse.bass as bass
import concourse.mybir as mybir

# ---------------------------------------------------------------- constants
N_NODES = 100_000
D_IN = 256
D_OUT = 128
N_EDGES = 1_600_000
N_CORES = 8

NB = 12_500          # nodes per block
NBP = 12_800         # padded block rows (25 * 512) - multiple of 512
NFB = 4              # from-blocks per core
NTB = 2              # to-blocks per core
NBUCKET = NFB * NTB  # 8 buckets per core

P1_ROWS = NFB * NBP  # 51200
P2_ROWS = NTB * NBP  # 25600

TILES1 = P1_ROWS // 128    # 400 node-tiles, table 1
TILES2 = P2_ROWS // 128    # 200 node-tiles, table 2
GROUPS1 = TILES1 // 4      # 100 psum groups
GROUPS2 = TILES2 // 4      # 50
NGROUP = GROUPS1 + GROUPS2  # 150
CHUNK_T = 20               # node-tiles per embT load chunk (= 5 groups)
NCH1 = TILES1 // CHUNK_T   # 20 chunks
NCH2 = TILES2 // CHUNK_T   # 10
NCHUNK = NCH1 + NCH2       # 30
EMB_COLS = CHUNK_T * 128   # 2560

MAX_CALL = 1024            # dma_gather idxs per call (HW limit; ring 4096/queue)

F32 = mybir.dt.float32
BF16 = mybir.dt.bfloat16
I16 = mybir.dt.int16
AX = mybir.AxisListType


# Projection block order: interleave p2/p1 blocks so gather buckets unblock
# as early as possible (bucket (fi,ti) needs p1 block fi + p2 block ti):
# (0,0)@50 groups, (0,1)@75, (1,*)@100, (2,*)@125, (3,*)@150.
BSEQ = [(1, 0), (0, 0), (1, 1), (0, 1), (0, 2), (0, 3)]
GPB = NBP // 512   # 25 groups per block
CPB = GPB // 5     # 5 chunks per block (CHUNK_T = 4 groups... 20 tiles)
GSEQ = []
CSEQ = []
for tab, blk in BSEQ:
    g0 = blk * GPB + (GROUPS1 if tab == 1 else 0)
    GSEQ += list(range(g0, g0 + GPB))
    c0 = blk * 5 + (NCH1 if tab == 1 else 0)
    CSEQ += list(range(c0, c0 + 5))
CPOS = {cid: q for q, cid in enumerate(CSEQ)}
_BPOS = {tb: i for i, tb in enumerate(BSEQ)}
# bucket (fi,ti) ready after this many groups in GSEQ order
BK_READY_Q = [GPB * (1 + max(_BPOS[(0, fi)], _BPOS[(1, ti)]))
              for fi in range(NFB) for ti in range(NTB)]


def _chunk_of_tile(tg):
    """global tile index -> (global chunk id, local col0 within chunk)."""
    if tg < TILES1:
        c = tg // CHUNK_T
        return c, (tg % CHUNK_T) * 128
    t2 = tg - TILES1
    c = NCH1 + t2 // CHUNK_T
    return c, (t2 % CHUNK_T) * 128


def _chunk_src(c):
    """global chunk id -> (table, col0)."""
    if c < NCH1:
        return 0, c * EMB_COLS
    return 1, (c - NCH1) * EMB_COLS


def _plan_calls(caps):
    """caps: per-bucket slot capacities (multiples of 128).
    Returns a list of gather-call PAIRS (bucket, n0, n1, idx_col0, slot_col0):
    two <=1024-idx gather calls whose outputs land adjacently so the DVE can
    process 2048 edges per op chain (n1 == 0 for a lone remainder call).
    Plus (idx_cols_total, slot_cols_total, per-bucket slot offsets)."""
    pairs = []
    icol = 0
    scol = 0
    boff = []
    for bk, cap in enumerate(caps):
        boff.append(scol)
        left = cap
        while left > 0:
            n0 = min(MAX_CALL, left)
            n1 = min(MAX_CALL, left - n0)
            pairs.append((bk, n0, n1, icol, scol))
            icol += (n0 + n1) // 16
            scol += (n0 + n1) // 128
            left -= n0 + n1
    return pairs, icol, scol, boff


# ---------------------------------------------------------------- device code
def build_bass(caps):
    """caps: per-bucket capacities (edge slots, multiples of 128), shared by
    all cores (max over cores)."""
    calls, idx_cols, tot_slots, boff = _plan_calls(caps)
    ncall = len(calls)

    nc = bass.Bass(dynamic_dma_scratch_size=65536, num_swdge_queues=2)

    e1t = nc.dram_tensor("e1t", [D_IN, P1_ROWS], BF16, kind="ExternalInput")
    e2t = nc.dram_tensor("e2t", [D_IN, P2_ROWS], BF16, kind="ExternalInput")
    w1 = nc.dram_tensor("w1", [D_IN, D_OUT], BF16, kind="ExternalInput")
    w2 = nc.dram_tensor("w2", [D_IN, D_OUT], BF16, kind="ExternalInput")
    b1r = nc.dram_tensor("b1r", [1, D_OUT], BF16, kind="ExternalInput")
    b2r = nc.dram_tensor("b2r", [1, D_OUT], BF16, kind="ExternalInput")
    onesr = nc.dram_tensor("onesr", [1, D_OUT], BF16, kind="ExternalInput")
    idxa = nc.dram_tensor("idxa", [128, idx_cols], I16, kind="ExternalInput")
    idxb = nc.dram_tensor("idxb", [128, idx_cols], I16, kind="ExternalInput")
    res = nc.dram_tensor("res", [128, tot_slots], F32, kind="ExternalOutput")

    p1d = nc.dram_tensor("p1d", [P1_ROWS, D_OUT], BF16, kind="Internal")
    p2d = nc.dram_tensor("p2d", [P2_ROWS, D_OUT], BF16, kind="Internal")
    pdst = (p1d, p2d)

    st = contextlib.ExitStack()
    with st:
        sb = lambda nm, shape, dt: st.enter_context(nc.sbuf_tensor(nm, shape, dt))
        sem = lambda nm: st.enter_context(nc.semaphore(name=nm))

        w1c = sb("w1c", [128, 256], BF16)
        w2c = sb("w2c", [128, 256], BF16)
        b1c = sb("b1c", [1, 128], BF16)
        b2c = sb("b2c", [1, 128], BF16)
        onc = sb("onc", [1, 128], BF16)
        idxt = (sb("idxta", [128, idx_cols], I16), sb("idxtb", [128, idx_cols], I16))
        et = [[sb(f"et_{p}_{k}", [128, EMB_COLS], BF16) for k in range(2)]
              for p in range(2)]  # [parity][k-half]
        pv = [sb(f"pv{i}", [128, 512], BF16) for i in range(4)]
        ps = [st.enter_context(nc.psum_tensor(f"ps{i}", [128, 512], F32))
              for i in range(4)]
        NBUF = 6  # gather-pair buffer rotation depth (5-stage DVE pipeline)
        at = [sb(f"at{i}", [128, 2 * MAX_CALL], BF16) for i in range(NBUF)]
        btg = [sb(f"btg{i}", [128, 2 * MAX_CALL], BF16) for i in range(NBUF)]
        rt = sb("rt", [128, tot_slots], F32)

        s_cl = sem("s_cl")               # w/b/ones const loads (7 dmas -> 112)
        s_cli = sem("s_cli")             # idx table loads (2 dmas -> 32)
        s_load = (sem("s_load0"), sem("s_load1"))  # embT loads, by chunk parity
        s_mm = sem("s_mm")               # matmuls (+1 each; 3 per tile)
        s_cp = sem("s_cp")               # act copies (+1 per group)
        s_pw = tuple(sem(f"s_pw{i}") for i in range(4))  # p-write dmas, by g%4
        # gather-completion sems by k%NBUF per queue: call k+NBUF's gathers are
        # gated on s_red >= k+1, so updates of each sem are causally ordered.
        s_ga = tuple(sem(f"s_ga{i}") for i in range(6))  # queue-0, +16/call
        s_gb = tuple(sem(f"s_gb{i}") for i in range(6))  # queue-1, +16/call
        s_st = tuple(sem(f"s_st{i}") for i in range(4))  # DVE dot stages 0-3
        s_red = sem("s_red")             # final reduces (+1 per call)
        s_out = sem("s_out")             # res dma

        CONSTS = 7 * 16

        block = st.enter_context(nc.Block())

        # ------------------------------------------------ SP: const + embT
        @block.sync
        def _(sync):
            for k in range(2):
                sync.dma_start(out=w1c[:, k * 128:(k + 1) * 128],
                               in_=w1[k * 128:(k + 1) * 128, :]).then_inc(s_cl, 16)
                sync.dma_start(out=w2c[:, k * 128:(k + 1) * 128],
                               in_=w2[k * 128:(k + 1) * 128, :]).then_inc(s_cl, 16)
            sync.dma_start(out=b1c[:], in_=b1r[:]).then_inc(s_cl, 16)
            sync.dma_start(out=b2c[:], in_=b2r[:]).then_inc(s_cl, 16)
            sync.dma_start(out=onc[:], in_=onesr[:]).then_inc(s_cl, 16)

            def load_chunk(cq):
                if cq >= 2:
                    # buffer cq%2 previously held chunk cq-2; wait consumed
                    sync.wait_ge(s_mm, 3 * CHUNK_T * (cq - 1))
                tab, col0 = _chunk_src(CSEQ[cq])
                src = e1t if tab == 0 else e2t
                par = cq % 2
                sync.dma_start(out=et[par][0][:],
                               in_=src[0:128, col0:col0 + EMB_COLS]).then_inc(s_load[par], 16)
                sync.dma_start(out=et[par][1][:],
                               in_=src[128:256, col0:col0 + EMB_COLS]).then_inc(s_load[par], 16)

            load_chunk(0)
            load_chunk(1)
            # idx tables aren't needed until the first gather bucket unblocks
            # (~50 groups in) - load them after the first two chunks so the PE
            # isn't starved at startup.
            sync.dma_start(out=idxt[0][:], in_=idxa[:]).then_inc(s_cli, 16)
            sync.dma_start(out=idxt[1][:], in_=idxb[:]).then_inc(s_cli, 16)
            next_cq = 2
            for q in range(NGROUP):
                # look ahead: issue loads for chunks starting within 3 groups
                while next_cq < NCHUNK and next_cq * CHUNK_T <= (q + 3) * 4 + 3:
                    load_chunk(next_cq)
                    next_cq += 1
                if next_cq >= NCHUNK:
                    break

        # ------------------------------------------------ PE: projections
        @block.tensor
        def _(tensor):
            tensor.wait_ge(s_cl, CONSTS)
            for q, g in enumerate(GSEQ):
                tab = 0 if g < GROUPS1 else 1
                wc = w1c if tab == 0 else w2c
                bc = b1c if tab == 0 else b2c
                for j in range(4):
                    tq = q * 4 + j
                    cid, col0 = _chunk_of_tile(g * 4 + j)
                    cq = CPOS[cid]
                    if tq == cq * CHUNK_T:  # first processed tile of chunk
                        tensor.wait_ge(s_load[cq % 2], 32 * (cq // 2 + 1))
                    if j == 0 and q >= 4:
                        tensor.wait_ge(s_cp, q - 3)  # psum bank q%4 free
                    out = ps[q % 4][:, j * 128:(j + 1) * 128]
                    tensor.matmul(out=out, lhsT=et[cq % 2][0][:, col0:col0 + 128],
                                  rhs=wc[:, 0:128], start=True, stop=False).then_inc(s_mm, 1)
                    tensor.matmul(out=out, lhsT=et[cq % 2][1][:, col0:col0 + 128],
                                  rhs=wc[:, 128:256], start=False, stop=False).then_inc(s_mm, 1)
                    tensor.matmul(out=out, lhsT=onc[:], rhs=bc[:],
                                  start=False, stop=True).then_inc(s_mm, 1)

        # ------------------------------------------------ Act: psum->sbuf + DMAs
        @block.scalar
        def _(scalar):
            def p_write(q):
                g = GSEQ[q]
                tab = 0 if g < GROUPS1 else 1
                r0 = g * 512 if tab == 0 else (g - GROUPS1) * 512
                # (p t)-permuted write: table row r0 + p*4 + t <- node r0 + t*128 + p
                scalar.dma_start(
                    out=pdst[tab][r0:r0 + 512, :].rearrange("(p t) d -> p t d", p=128),
                    in_=pv[q % 4][:].rearrange("p (t d) -> p t d", d=128),
                ).then_inc(s_pw[q % 4], 16)

            scalar.wait_ge(s_cl, CONSTS)
            # p-write of group q-1 is issued after the copy of group q so the
            # copy-completion sem is already visible when its write issues.
            for q, g in enumerate(GSEQ):
                scalar.wait_ge(s_mm, 12 * q + 12)
                if q >= 4:
                    scalar.wait_ge(s_pw[q % 4], 16 * (q // 4))  # pv[q%4] drained
                scalar.copy(out=pv[q % 4][:], in_=ps[q % 4][:]).then_inc(s_cp, 1)
                if q >= 1:
                    scalar.wait_ge(s_cp, q)
                    p_write(q - 1)
            scalar.wait_ge(s_cp, NGROUP)
            p_write(NGROUP - 1)
            scalar.wait_ge(s_red, ncall)
            scalar.dma_start(out=res[:], in_=rt[:]).then_inc(s_out, 16)
            scalar.wait_ge(s_out, 16)

        # ------------------------------------------------ Pool: gathers
        @block.gpsimd
        def _(gpsimd):
            from concourse import library_config
            gpsimd.load_library(library_config.mlp)
            regs = {n: gpsimd.to_reg(n) for n in sorted({c[1] for c in calls})}
            gpsimd.wait_ge(s_cl, CONSTS)
            gpsimd.wait_ge(s_cli, 32)
            gated_bk = -1
            for p, (bk, n0, n1, icol, scol) in enumerate(calls):
                fi, ti = bk // NTB, bk % NTB
                if bk > gated_bk:
                    gated_bk = bk
                    nq = BK_READY_Q[bk]
                    for r in range(4):
                        gpsimd.wait_ge(s_pw[r], 16 * len(range(r, nq, 4)))
                if p >= NBUF:
                    gpsimd.wait_ge(s_red, p - NBUF + 1)  # at/btg[p%NBUF] consumed
                # rows are moved as 64 packed f32 (= 128 bf16): same bytes and
                # descriptors, half the modeled element count.
                nh = 0
                for h, n in enumerate((n0, n1)):
                    if n == 0:
                        continue
                    S = n // 128
                    c0 = icol + nh // 16
                    gpsimd.dma_gather(
                        out_ap=at[p % NBUF][:, nh:nh + S * 128].bitcast(F32)
                            .rearrange("p (s d) -> p s d", d=64),
                        in_ap=p1d[fi * NBP:(fi + 1) * NBP, :].bitcast(F32),
                        idxs_ap=idxt[0][:, c0:c0 + n // 16],
                        num_idxs=n, num_idxs_reg=regs[n], elem_size=D_OUT // 2,
                        queue_num=0,
                    ).then_inc(s_ga[p % NBUF], 16)
                    gpsimd.dma_gather(
                        out_ap=btg[p % NBUF][:, nh:nh + S * 128].bitcast(F32)
                            .rearrange("p (s d) -> p s d", d=64),
                        in_ap=p2d[ti * NBP:(ti + 1) * NBP, :].bitcast(F32),
                        idxs_ap=idxt[1][:, c0:c0 + n // 16],
                        num_idxs=n, num_idxs_reg=regs[n], elem_size=D_OUT // 2,
                        queue_num=1,
                    ).then_inc(s_gb[p % NBUF], 16)
                    nh += S * 128

        # ------------------------------------------------ DVE: dot products
        # bf16 multiply (2x mode), tree-halve in bf16 (2x) down to 16 partials
        # per slot, then one short f32 reduce.  The 5-stage chain is software-
        # pipelined across calls (stage s of call k in round k+s) so every
        # intra-chain semaphore is already visible when its wait issues.
        @block.vector
        def _(vector):
            def stage(s, k):
                bk, n, icol, scol = calls[k]
                S = n // 128
                av = at[k % 6][:, :S * 128].rearrange("p (s d) -> p s d", d=128)
                if s == 0:
                    vector.wait_ge(s_ga[k % 6], 16 * (k // 6 + 1))
                    vector.wait_ge(s_gb[k % 6], 16 * (k // 6 + 1))
                    a3 = at[k % 6][:, :S * 128]
                    vector.tensor_mul(out=a3, in0=a3,
                                      in1=btg[k % 6][:, :S * 128]).then_inc(s_st[0], 1)
                elif s in (1, 2, 3):
                    h = 128 >> s  # 64, 32, 16
                    vector.wait_ge(s_st[s - 1], k + 1)
                    vector.tensor_add(out=av[:, :, 0:h], in0=av[:, :, 0:h],
                                      in1=av[:, :, h:2 * h]).then_inc(s_st[s], 1)
                else:
                    vector.wait_ge(s_st[3], k + 1)
                    vector.reduce_sum(out=rt[:, scol:scol + S],
                                      in_=av[:, :, 0:16], axis=AX.X).then_inc(s_red, 1)

            for r in range(ncall + 4):
                for s in range(4, -1, -1):
                    k = r - s
                    if 0 <= k < ncall:
                        stage(s, k)

    return nc, calls, boff


_NC_CACHE: dict = {}


def _get_nc(caps):
    key = tuple(caps)
    if key not in _NC_CACHE:
        nc, calls, boff = build_bass(caps)
        from concourse.library_overlay import lower_extended_insts
        lower_extended_insts(nc)
        _NC_CACHE[key] = (nc, calls, boff)
    return _NC_CACHE[key]


# ---------------------------------------------------------------- host side
def _perm_local(n):
    """block-local node id -> permuted table row (within block).
    Table row g*512 + p*4 + t holds node g*512 + t*128 + p."""
    g, u = np.divmod(n, 512)
    t, p = np.divmod(u, 128)
    return g * 512 + p * 4 + t


def _marshal(emb_1, emb_2, nodes_from_to, W1, b1, W2, b2):
    """Shard/bucket inputs per core.  Returns (caps, in_maps, books)."""
    import ml_dtypes
    bf16 = ml_dtypes.bfloat16

    f = np.asarray(nodes_from_to[:, 0], dtype=np.int64)
    t = np.asarray(nodes_from_to[:, 1], dtype=np.int64)
    emb_1 = np.asarray(emb_1, dtype=np.float32)
    emb_2 = np.asarray(emb_2, dtype=np.float32)
    W1 = np.asarray(W1, dtype=np.float32).astype(bf16)
    W2 = np.asarray(W2, dtype=np.float32).astype(bf16)
    b1 = np.asarray(b1, dtype=np.float32).reshape(1, -1).astype(bf16)
    b2 = np.asarray(b2, dtype=np.float32).reshape(1, -1).astype(bf16)
    onesr = np.ones((1, D_OUT), np.float32).astype(bf16)

    core = (f // (NFB * NB)) * 4 + t // (NTB * NB)
    order0 = np.argsort(core, kind="stable")
    ccnt = np.bincount(core, minlength=N_CORES)
    coff = np.concatenate([[0], np.cumsum(ccnt)])

    percore = []
    all_cnts = np.zeros((N_CORES, NBUCKET), np.int64)
    for c in range(N_CORES):
        a, b = c // 4, c % 4
        sel = order0[coff[c]:coff[c + 1]]
        fc, tcv = f[sel], t[sel]
        fi = fc // NB - NFB * a
        ti = tcv // NB - NTB * b
        fl = _perm_local(fc % NB).astype(np.int16)
        tl = _perm_local(tcv % NB).astype(np.int16)
        bk = fi * NTB + ti
        o2 = np.argsort(bk, kind="stable")
        sel2, fl2, tl2 = sel[o2], fl[o2], tl[o2]
        cnts = np.bincount(bk, minlength=NBUCKET)
        all_cnts[c] = cnts
        percore.append((a, b, sel2, fl2, tl2, cnts))

    caps = [int(-(-all_cnts[:, k].max() // 128) * 128) for k in range(NBUCKET)]
    calls, idx_cols, tot_slots, boff = _plan_calls(caps)

    in_maps, books = [], []
    for c in range(N_CORES):
        a, b, sel2, fl2, tl2, cnts = percore[c]
        pos = np.concatenate([[0], np.cumsum(cnts)])

        slots_a = np.zeros((NBUCKET, max(caps)), np.int16)
        slots_b = np.zeros((NBUCKET, max(caps)), np.int16)
        for k in range(NBUCKET):
            slots_a[k, :cnts[k]] = fl2[pos[k]:pos[k + 1]]
            slots_b[k, :cnts[k]] = tl2[pos[k]:pos[k + 1]]
        # wrap by 16: idx i of a bucket at (partition i%16, col i//16),
        # replicated across the 8 groups of 16 partitions
        wa_cols = []
        wb_cols = []
        for k in range(NBUCKET):
            cap = caps[k]
            wa_cols.append(slots_a[k, :cap].reshape(cap // 16, 16).T)
            wb_cols.append(slots_b[k, :cap].reshape(cap // 16, 16).T)
        idxa = np.tile(np.concatenate(wa_cols, axis=1), (8, 1))
        idxb = np.tile(np.concatenate(wb_cols, axis=1), (8, 1))

        e1t = np.zeros((D_IN, P1_ROWS), bf16)
        for i in range(NFB):
            blk = emb_1[(NFB * a + i) * NB:(NFB * a + i + 1) * NB]
            e1t[:, i * NBP:i * NBP + NB] = blk.T.astype(bf16)
        e2t = np.zeros((D_IN, P2_ROWS), bf16)
        for i in range(NTB):
            blk = emb_2[(NTB * b + i) * NB:(NTB * b + i + 1) * NB]
            e2t[:, i * NBP:i * NBP + NB] = blk.T.astype(bf16)

        in_maps.append({
            "e1t": e1t, "e2t": e2t,
            "w1": W1, "w2": W2, "b1r": b1, "b2r": b2, "onesr": onesr,
            "idxa": np.ascontiguousarray(idxa),
            "idxb": np.ascontiguousarray(idxb),
        })
        books.append((sel2, cnts, pos))
    return caps, in_maps, books


def _unmarshal(results, books, caps, n_edges):
    calls, idx_cols, tot_slots, boff = _plan_calls(caps)
    out = np.empty(n_edges, np.float32)
    for c in range(N_CORES):
        sel2, cnts, pos = books[c]
        r = results[c]["res"]  # [128, tot_slots]
        for k in range(NBUCKET):
            if cnts[k] == 0:
                continue
            s0 = boff[k]
            nslots = caps[k] // 128
            stream = r[:, s0:s0 + nslots].T.reshape(-1)
            out[sel2[pos[k]:pos[k + 1]]] = stream[:cnts[k]]
    return out


def _run(inputs, trace=False, **run_kwargs):
    from concourse.bass_utils import run_bass_kernel_spmd

    caps, in_maps, books = _marshal(**inputs)
    nc, calls, boff = _get_nc(caps)
    r = run_bass_kernel_spmd(
        nc, in_maps, core_ids=list(range(N_CORES)), trace=trace, **run_kwargs
    )
    out = _unmarshal(r.results, books, caps, len(inputs["nodes_from_to"]))
    return out, r


def kernel(**inputs) -> np.ndarray:
    out, _ = _run(inputs, trace=False)
    return out


# revision 23
# speedup vs baseline: 2.1594x; 1.0784x over previous
"""Trainium2 Bass kernel for nn_BetweenClusterFC.

Computes out[e] = (emb_1[f[e]] @ W1 + b1) . (emb_2[t[e]] @ W2 + b2)
for E = 1.6M edges over N = 100k nodes, D_IN = 256, D_OUT = 128.

Strategy (8 NeuronCores, SPMD, full inputs in / full output out):
  - Nodes are split into 8 blocks of 12500.  Edges are assigned to cores by a
    (from-block-group, to-block-group) 4x2 rectangle: core c=(a,b) handles
    edges with from-node in blocks [4a..4a+3] and to-node in blocks
    [2b..2b+1].  Uniform (~200k edges/core); each core needs projections for
    4 from-blocks + 2 to-blocks (75k nodes).
  - All streamed data is bf16: the host pre-transposes and casts the
    embedding shards; the PE projects p = emb @ W (+ bias folded in as a
    K=1 ones x bias matmul), the Act engine moves psum -> sbuf in bf16 and
    issues the p-table writes (1KB-contiguous via a (p t)-permuted row
    order), keeping the DVE entirely free for the edge phase.
  - Edges are bucketed host-side by (local from-block, local to-block); per
    bucket both endpoint rows are fetched with SWDGE dma_gather calls (up to
    2048 idxs/call, two queues) using int16 permuted-local indices; a DVE
    bf16 multiply (2x mode) + f32 reduce produces the per-edge dots into a
    single SBUF-resident result tile, written out once at the end.
  - Bucket capacities are sized from the actual per-core edge counts (max
    across cores, 128-aligned), so the Bass program is built per problem
    instance (inspector-executor style) and cached.
  - The host applies the inverse edge permutation to assemble the output.

Written in raw Bass (explicit semaphores) - the Tile layer's generated sync
exceeds this toolchain's per-instruction wait-slot limits.
"""

import contextlib

import numpy as np

import concourse.bass as bass
import concourse.mybir as mybir

# ---------------------------------------------------------------- constants
N_NODES = 100_000
D_IN = 256
D_OUT = 128
N_EDGES = 1_600_000
N_CORES = 8

NB = 12_500          # nodes per block
NBP = 12_800         # padded block rows (25 * 512) - multiple of 512
NFB = 4              # from-blocks per core
NTB = 2              # to-blocks per core
NBUCKET = NFB * NTB  # 8 buckets per core

P1_ROWS = NFB * NBP  # 51200
P2_ROWS = NTB * NBP  # 25600

TILES1 = P1_ROWS // 128    # 400 node-tiles, table 1
TILES2 = P2_ROWS // 128    # 200 node-tiles, table 2
GROUPS1 = TILES1 // 4      # 100 psum groups
GROUPS2 = TILES2 // 4      # 50
NGROUP = GROUPS1 + GROUPS2  # 150
CHUNK_T = 20               # node-tiles per embT load chunk (= 5 groups)
NCH1 = TILES1 // CHUNK_T   # 20 chunks
NCH2 = TILES2 // CHUNK_T   # 10
NCHUNK = NCH1 + NCH2       # 30
EMB_COLS = CHUNK_T * 128   # 2560

MAX_CALL = 1024            # dma_gather idxs per call (HW limit; ring 4096/queue)

F32 = mybir.dt.float32
BF16 = mybir.dt.bfloat16
I16 = mybir.dt.int16
AX = mybir.AxisListType


# Projection block order: interleave p2/p1 blocks so gather buckets unblock
# as early as possible (bucket (fi,ti) needs p1 block fi + p2 block ti):
# (0,0)@50 groups, (0,1)@75, (1,*)@100, (2,*)@125, (3,*)@150.
BSEQ = [(1, 0), (0, 0), (1, 1), (0, 1), (0, 2), (0, 3)]
GPB = NBP // 512   # 25 groups per block
CPB = GPB // 5     # 5 chunks per block (CHUNK_T = 4 groups... 20 tiles)
GSEQ = []
CSEQ = []
for tab, blk in BSEQ:
    g0 = blk * GPB + (GROUPS1 if tab == 1 else 0)
    GSEQ += list(range(g0, g0 + GPB))
    c0 = blk * 5 + (NCH1 if tab == 1 else 0)
    CSEQ += list(range(c0, c0 + 5))
CPOS = {cid: q for q, cid in enumerate(CSEQ)}
_BPOS = {tb: i for i, tb in enumerate(BSEQ)}
# bucket (fi,ti) ready after this many groups in GSEQ order
BK_READY_Q = [GPB * (1 + max(_BPOS[(0, fi)], _BPOS[(1, ti)]))
              for fi in range(NFB) for ti in range(NTB)]


def _chunk_of_tile(tg):
    """global tile index -> (global chunk id, local col0 within chunk)."""
    if tg < TILES1:
        c = tg // CHUNK_T
        return c, (tg % CHUNK_T) * 128
    t2 = tg - TILES1
    c = NCH1 + t2 // CHUNK_T
    return c, (t2 % CHUNK_T) * 128


def _chunk_src(c):
    """global chunk id -> (table, col0)."""
    if c < NCH1:
        return 0, c * EMB_COLS
    return 1, (c - NCH1) * EMB_COLS


def _plan_calls(caps):
    """caps: per-bucket slot capacities (multiples of 128).
    Returns a list of gather-call PAIRS (bucket, n0, n1, idx_col0, slot_col0):
    two <=1024-idx gather calls whose outputs land adjacently so the DVE can
    process 2048 edges per op chain (n1 == 0 for a lone remainder call).
    Plus (idx_cols_total, slot_cols_total, per-bucket slot offsets)."""
    pairs = []
    icol = 0
    scol = 0
    boff = []
    for bk, cap in enumerate(caps):
        boff.append(scol)
        left = cap
        while left > 0:
            n0 = min(MAX_CALL, left)
            n1 = min(MAX_CALL, left - n0)
            pairs.append((bk, n0, n1, icol, scol))
            icol += (n0 + n1) // 16
            scol += (n0 + n1) // 128
            left -= n0 + n1
    return pairs, icol, scol, boff


# ---------------------------------------------------------------- device code
def build_bass(caps):
    """caps: per-bucket capacities (edge slots, multiples of 128), shared by
    all cores (max over cores)."""
    calls, idx_cols, tot_slots, boff = _plan_calls(caps)
    ncall = len(calls)

    nc = bass.Bass(dynamic_dma_scratch_size=65536, num_swdge_queues=2)

    e1t = nc.dram_tensor("e1t", [D_IN, P1_ROWS], BF16, kind="ExternalInput")
    e2t = nc.dram_tensor("e2t", [D_IN, P2_ROWS], BF16, kind="ExternalInput")
    w1 = nc.dram_tensor("w1", [D_IN, D_OUT], BF16, kind="ExternalInput")
    w2 = nc.dram_tensor("w2", [D_IN, D_OUT], BF16, kind="ExternalInput")
    b1r = nc.dram_tensor("b1r", [1, D_OUT], BF16, kind="ExternalInput")
    b2r = nc.dram_tensor("b2r", [1, D_OUT], BF16, kind="ExternalInput")
    onesr = nc.dram_tensor("onesr", [1, D_OUT], BF16, kind="ExternalInput")
    idxa = nc.dram_tensor("idxa", [128, idx_cols], I16, kind="ExternalInput")
    idxb = nc.dram_tensor("idxb", [128, idx_cols], I16, kind="ExternalInput")
    res = nc.dram_tensor("res", [128, tot_slots], F32, kind="ExternalOutput")

    p1d = nc.dram_tensor("p1d", [P1_ROWS, D_OUT], BF16, kind="Internal")
    p2d = nc.dram_tensor("p2d", [P2_ROWS, D_OUT], BF16, kind="Internal")
    pdst = (p1d, p2d)

    st = contextlib.ExitStack()
    with st:
        sb = lambda nm, shape, dt: st.enter_context(nc.sbuf_tensor(nm, shape, dt))
        sem = lambda nm: st.enter_context(nc.semaphore(name=nm))

        w1c = sb("w1c", [128, 256], BF16)
        w2c = sb("w2c", [128, 256], BF16)
        b1c = sb("b1c", [1, 128], BF16)
        b2c = sb("b2c", [1, 128], BF16)
        onc = sb("onc", [1, 128], BF16)
        idxt = (sb("idxta", [128, idx_cols], I16), sb("idxtb", [128, idx_cols], I16))
        et = [[sb(f"et_{p}_{k}", [128, EMB_COLS], BF16) for k in range(2)]
              for p in range(2)]  # [parity][k-half]
        pv = [sb(f"pv{i}", [128, 512], BF16) for i in range(4)]
        ps = [st.enter_context(nc.psum_tensor(f"ps{i}", [128, 512], F32))
              for i in range(4)]
        NBUF = 6  # gather-pair buffer rotation depth (5-stage DVE pipeline)
        at = [sb(f"at{i}", [128, 2 * MAX_CALL], BF16) for i in range(NBUF)]
        btg = [sb(f"btg{i}", [128, 2 * MAX_CALL], BF16) for i in range(NBUF)]
        rt = sb("rt", [128, tot_slots], F32)

        s_cl = sem("s_cl")               # w/b/ones const loads (7 dmas -> 112)
        s_cli = sem("s_cli")             # idx table loads (2 dmas -> 32)
        s_load = (sem("s_load0"), sem("s_load1"))  # embT loads, by chunk parity
        s_mm = sem("s_mm")               # matmuls (+1 each; 3 per tile)
        s_cp = sem("s_cp")               # act copies (+1 per group)
        s_pw = tuple(sem(f"s_pw{i}") for i in range(4))  # p-write dmas, by g%4
        # gather-completion sems by k%NBUF per queue: call k+NBUF's gathers are
        # gated on s_red >= k+1, so updates of each sem are causally ordered.
        s_ga = tuple(sem(f"s_ga{i}") for i in range(6))  # queue-0, +16/call
        s_gb = tuple(sem(f"s_gb{i}") for i in range(6))  # queue-1, +16/call
        s_st = tuple(sem(f"s_st{i}") for i in range(4))  # DVE dot stages 0-3
        s_red = sem("s_red")             # final reduces (+1 per call)
        s_out = sem("s_out")             # res dma

        CONSTS = 7 * 16

        block = st.enter_context(nc.Block())

        # ------------------------------------------------ SP: const + embT
        @block.sync
        def _(sync):
            for k in range(2):
                sync.dma_start(out=w1c[:, k * 128:(k + 1) * 128],
                               in_=w1[k * 128:(k + 1) * 128, :]).then_inc(s_cl, 16)
                sync.dma_start(out=w2c[:, k * 128:(k + 1) * 128],
                               in_=w2[k * 128:(k + 1) * 128, :]).then_inc(s_cl, 16)
            sync.dma_start(out=b1c[:], in_=b1r[:]).then_inc(s_cl, 16)
            sync.dma_start(out=b2c[:], in_=b2r[:]).then_inc(s_cl, 16)
            sync.dma_start(out=onc[:], in_=onesr[:]).then_inc(s_cl, 16)

            def load_chunk(cq):
                if cq >= 2:
                    # buffer cq%2 previously held chunk cq-2; wait consumed
                    sync.wait_ge(s_mm, 3 * CHUNK_T * (cq - 1))
                tab, col0 = _chunk_src(CSEQ[cq])
                src = e1t if tab == 0 else e2t
                par = cq % 2
                sync.dma_start(out=et[par][0][:],
                               in_=src[0:128, col0:col0 + EMB_COLS]).then_inc(s_load[par], 16)
                sync.dma_start(out=et[par][1][:],
                               in_=src[128:256, col0:col0 + EMB_COLS]).then_inc(s_load[par], 16)

            load_chunk(0)
            load_chunk(1)
            # idx tables aren't needed until the first gather bucket unblocks
            # (~50 groups in) - load them after the first two chunks so the PE
            # isn't starved at startup.
            sync.dma_start(out=idxt[0][:], in_=idxa[:]).then_inc(s_cli, 16)
            sync.dma_start(out=idxt[1][:], in_=idxb[:]).then_inc(s_cli, 16)
            next_cq = 2
            for q in range(NGROUP):
                # look ahead: issue loads for chunks starting within 3 groups
                while next_cq < NCHUNK and next_cq * CHUNK_T <= (q + 3) * 4 + 3:
                    load_chunk(next_cq)
                    next_cq += 1
                if next_cq >= NCHUNK:
                    break

        # ------------------------------------------------ PE: projections
        @block.tensor
        def _(tensor):
            tensor.wait_ge(s_cl, CONSTS)
            for q, g in enumerate(GSEQ):
                tab = 0 if g < GROUPS1 else 1
                wc = w1c if tab == 0 else w2c
                bc = b1c if tab == 0 else b2c
                for j in range(4):
                    tq = q * 4 + j
                    cid, col0 = _chunk_of_tile(g * 4 + j)
                    cq = CPOS[cid]
                    if tq == cq * CHUNK_T:  # first processed tile of chunk
                        tensor.wait_ge(s_load[cq % 2], 32 * (cq // 2 + 1))
                    if j == 0 and q >= 4:
                        tensor.wait_ge(s_cp, q - 3)  # psum bank q%4 free
                    out = ps[q % 4][:, j * 128:(j + 1) * 128]
                    tensor.matmul(out=out, lhsT=et[cq % 2][0][:, col0:col0 + 128],
                                  rhs=wc[:, 0:128], start=True, stop=False).then_inc(s_mm, 1)
                    tensor.matmul(out=out, lhsT=et[cq % 2][1][:, col0:col0 + 128],
                                  rhs=wc[:, 128:256], start=False, stop=False).then_inc(s_mm, 1)
                    tensor.matmul(out=out, lhsT=onc[:], rhs=bc[:],
                                  start=False, stop=True).then_inc(s_mm, 1)

        # ------------------------------------------------ Act: psum->sbuf + DMAs
        @block.scalar
        def _(scalar):
            def p_write(q):
                g = GSEQ[q]
                tab = 0 if g < GROUPS1 else 1
                r0 = g * 512 if tab == 0 else (g - GROUPS1) * 512
                # (p t)-permuted write: table row r0 + p*4 + t <- node r0 + t*128 + p
                scalar.dma_start(
                    out=pdst[tab][r0:r0 + 512, :].rearrange("(p t) d -> p t d", p=128),
                    in_=pv[q % 4][:].rearrange("p (t d) -> p t d", d=128),
                ).then_inc(s_pw[q % 4], 16)

            scalar.wait_ge(s_cl, CONSTS)
            # p-write of group q-1 is issued after the copy of group q so the
            # copy-completion sem is already visible when its write issues.
            for q, g in enumerate(GSEQ):
                scalar.wait_ge(s_mm, 12 * q + 12)
                if q >= 4:
                    scalar.wait_ge(s_pw[q % 4], 16 * (q // 4))  # pv[q%4] drained
                scalar.copy(out=pv[q % 4][:], in_=ps[q % 4][:]).then_inc(s_cp, 1)
                if q >= 1:
                    scalar.wait_ge(s_cp, q)
                    p_write(q - 1)
            scalar.wait_ge(s_cp, NGROUP)
            p_write(NGROUP - 1)
            scalar.wait_ge(s_red, ncall)
            scalar.dma_start(out=res[:], in_=rt[:]).then_inc(s_out, 16)
            scalar.wait_ge(s_out, 16)

        # ------------------------------------------------ Pool: gathers
        @block.gpsimd
        def _(gpsimd):
            from concourse import library_config
            gpsimd.load_library(library_config.mlp)
            sizes = ({c[1] for c in calls} | {c[2] for c in calls}) - {0}
            regs = {n: gpsimd.to_reg(n) for n in sorted(sizes)}
            gpsimd.wait_ge(s_cl, CONSTS)
            gpsimd.wait_ge(s_cli, 32)
            gated_bk = -1
            for p, (bk, n0, n1, icol, scol) in enumerate(calls):
                fi, ti = bk // NTB, bk % NTB
                if bk > gated_bk:
                    gated_bk = bk
                    nq = BK_READY_Q[bk]
                    for r in range(4):
                        gpsimd.wait_ge(s_pw[r], 16 * len(range(r, nq, 4)))
                if p >= NBUF:
                    gpsimd.wait_ge(s_red, p - NBUF + 1)  # at/btg[p%NBUF] consumed
                # rows are moved as 64 packed f32 (= 128 bf16): same bytes and
                # descriptors, half the modeled element count.
                nh = 0
                for h, n in enumerate((n0, n1)):
                    if n == 0:
                        continue
                    S = n // 128
                    c0 = icol + nh // 16
                    gpsimd.dma_gather(
                        out_ap=at[p % NBUF][:, nh:nh + S * 128].bitcast(F32)
                            .rearrange("p (s d) -> p s d", d=64),
                        in_ap=p1d[fi * NBP:(fi + 1) * NBP, :].bitcast(F32),
                        idxs_ap=idxt[0][:, c0:c0 + n // 16],
                        num_idxs=n, num_idxs_reg=regs[n], elem_size=D_OUT // 2,
                        queue_num=0,
                    ).then_inc(s_ga[p % NBUF], 16)
                    gpsimd.dma_gather(
                        out_ap=btg[p % NBUF][:, nh:nh + S * 128].bitcast(F32)
                            .rearrange("p (s d) -> p s d", d=64),
                        in_ap=p2d[ti * NBP:(ti + 1) * NBP, :].bitcast(F32),
                        idxs_ap=idxt[1][:, c0:c0 + n // 16],
                        num_idxs=n, num_idxs_reg=regs[n], elem_size=D_OUT // 2,
                        queue_num=1,
                    ).then_inc(s_gb[p % NBUF], 16)
                    nh += S * 128

        # ------------------------------------------------ DVE: dot products
        # bf16 multiply (2x mode), tree-halve in bf16 (2x) down to 16 partials
        # per slot, then one short f32 reduce - one chain per gather pair
        # (up to 2048 edges).  The 5-stage chain is software-pipelined across
        # pairs (stage s of pair p in round p+s) so every intra-chain
        # semaphore is already visible when its wait issues.
        @block.vector
        def _(vector):
            # per-sem cumulative gather counts: pair p incs s_g*[p%NBUF] by
            # 16 per issued call (2 for a full pair, 1 for a lone remainder)
            sem_cnt = [0] * NBUF
            gwait = []
            for p2, (bk, n0, n1, icol, scol) in enumerate(calls):
                sem_cnt[p2 % NBUF] += 16 * (2 if n1 else 1)
                gwait.append(sem_cnt[p2 % NBUF])

            def stage(s, p):
                bk, n0, n1, icol, scol = calls[p]
                S = (n0 + n1) // 128
                av = at[p % NBUF][:, :S * 128].rearrange("p (s d) -> p s d", d=128)
                if s == 0:
                    vector.wait_ge(s_ga[p % NBUF], gwait[p])
                    vector.wait_ge(s_gb[p % NBUF], gwait[p])
                    a3 = at[p % NBUF][:, :S * 128]
                    vector.tensor_mul(out=a3, in0=a3,
                                      in1=btg[p % NBUF][:, :S * 128]).then_inc(s_st[0], 1)
                elif s in (1, 2, 3):
                    h = 128 >> s  # 64, 32, 16
                    vector.wait_ge(s_st[s - 1], p + 1)
                    vector.tensor_add(out=av[:, :, 0:h], in0=av[:, :, 0:h],
                                      in1=av[:, :, h:2 * h]).then_inc(s_st[s], 1)
                else:
                    vector.wait_ge(s_st[3], p + 1)
                    vector.reduce_sum(out=rt[:, scol:scol + S],
                                      in_=av[:, :, 0:16], axis=AX.X).then_inc(s_red, 1)

            for r in range(ncall + 4):
                for s in range(4, -1, -1):
                    p = r - s
                    if 0 <= p < ncall:
                        stage(s, p)

    return nc, calls, boff


_NC_CACHE: dict = {}


def _get_nc(caps):
    key = tuple(caps)
    if key not in _NC_CACHE:
        nc, calls, boff = build_bass(caps)
        from concourse.library_overlay import lower_extended_insts
        lower_extended_insts(nc)
        _NC_CACHE[key] = (nc, calls, boff)
    return _NC_CACHE[key]


# ---------------------------------------------------------------- host side
def _perm_local(n):
    """block-local node id -> permuted table row (within block).
    Table row g*512 + p*4 + t holds node g*512 + t*128 + p."""
    g, u = np.divmod(n, 512)
    t, p = np.divmod(u, 128)
    return g * 512 + p * 4 + t


def _marshal(emb_1, emb_2, nodes_from_to, W1, b1, W2, b2):
    """Shard/bucket inputs per core.  Returns (caps, in_maps, books)."""
    import ml_dtypes
    bf16 = ml_dtypes.bfloat16

    f = np.asarray(nodes_from_to[:, 0], dtype=np.int64)
    t = np.asarray(nodes_from_to[:, 1], dtype=np.int64)
    emb_1 = np.asarray(emb_1, dtype=np.float32)
    emb_2 = np.asarray(emb_2, dtype=np.float32)
    W1 = np.asarray(W1, dtype=np.float32).astype(bf16)
    W2 = np.asarray(W2, dtype=np.float32).astype(bf16)
    b1 = np.asarray(b1, dtype=np.float32).reshape(1, -1).astype(bf16)
    b2 = np.asarray(b2, dtype=np.float32).reshape(1, -1).astype(bf16)
    onesr = np.ones((1, D_OUT), np.float32).astype(bf16)

    core = (f // (NFB * NB)) * 4 + t // (NTB * NB)
    order0 = np.argsort(core, kind="stable")
    ccnt = np.bincount(core, minlength=N_CORES)
    coff = np.concatenate([[0], np.cumsum(ccnt)])

    percore = []
    all_cnts = np.zeros((N_CORES, NBUCKET), np.int64)
    for c in range(N_CORES):
        a, b = c // 4, c % 4
        sel = order0[coff[c]:coff[c + 1]]
        fc, tcv = f[sel], t[sel]
        fi = fc // NB - NFB * a
        ti = tcv // NB - NTB * b
        fl = _perm_local(fc % NB).astype(np.int16)
        tl = _perm_local(tcv % NB).astype(np.int16)
        bk = fi * NTB + ti
        o2 = np.argsort(bk, kind="stable")
        sel2, fl2, tl2 = sel[o2], fl[o2], tl[o2]
        cnts = np.bincount(bk, minlength=NBUCKET)
        all_cnts[c] = cnts
        percore.append((a, b, sel2, fl2, tl2, cnts))

    caps = [int(-(-all_cnts[:, k].max() // 128) * 128) for k in range(NBUCKET)]
    calls, idx_cols, tot_slots, boff = _plan_calls(caps)

    in_maps, books = [], []
    for c in range(N_CORES):
        a, b, sel2, fl2, tl2, cnts = percore[c]
        pos = np.concatenate([[0], np.cumsum(cnts)])

        slots_a = np.zeros((NBUCKET, max(caps)), np.int16)
        slots_b = np.zeros((NBUCKET, max(caps)), np.int16)
        for k in range(NBUCKET):
            slots_a[k, :cnts[k]] = fl2[pos[k]:pos[k + 1]]
            slots_b[k, :cnts[k]] = tl2[pos[k]:pos[k + 1]]
        # wrap by 16: idx i of a bucket at (partition i%16, col i//16),
        # replicated across the 8 groups of 16 partitions
        wa_cols = []
        wb_cols = []
        for k in range(NBUCKET):
            cap = caps[k]
            wa_cols.append(slots_a[k, :cap].reshape(cap // 16, 16).T)
            wb_cols.append(slots_b[k, :cap].reshape(cap // 16, 16).T)
        idxa = np.tile(np.concatenate(wa_cols, axis=1), (8, 1))
        idxb = np.tile(np.concatenate(wb_cols, axis=1), (8, 1))

        e1t = np.zeros((D_IN, P1_ROWS), bf16)
        for i in range(NFB):
            blk = emb_1[(NFB * a + i) * NB:(NFB * a + i + 1) * NB]
            e1t[:, i * NBP:i * NBP + NB] = blk.T.astype(bf16)
        e2t = np.zeros((D_IN, P2_ROWS), bf16)
        for i in range(NTB):
            blk = emb_2[(NTB * b + i) * NB:(NTB * b + i + 1) * NB]
            e2t[:, i * NBP:i * NBP + NB] = blk.T.astype(bf16)

        in_maps.append({
            "e1t": e1t, "e2t": e2t,
            "w1": W1, "w2": W2, "b1r": b1, "b2r": b2, "onesr": onesr,
            "idxa": np.ascontiguousarray(idxa),
            "idxb": np.ascontiguousarray(idxb),
        })
        books.append((sel2, cnts, pos))
    return caps, in_maps, books


def _unmarshal(results, books, caps, n_edges):
    calls, idx_cols, tot_slots, boff = _plan_calls(caps)
    out = np.empty(n_edges, np.float32)
    for c in range(N_CORES):
        sel2, cnts, pos = books[c]
        r = results[c]["res"]  # [128, tot_slots]
        for k in range(NBUCKET):
            if cnts[k] == 0:
                continue
            s0 = boff[k]
            nslots = caps[k] // 128
            stream = r[:, s0:s0 + nslots].T.reshape(-1)
            out[sel2[pos[k]:pos[k + 1]]] = stream[:cnts[k]]
    return out


def _run(inputs, trace=False, **run_kwargs):
    from concourse.bass_utils import run_bass_kernel_spmd

    caps, in_maps, books = _marshal(**inputs)
    nc, calls, boff = _get_nc(caps)
    r = run_bass_kernel_spmd(
        nc, in_maps, core_ids=list(range(N_CORES)), trace=trace, **run_kwargs
    )
    out = _unmarshal(r.results, books, caps, len(inputs["nodes_from_to"]))
    return out, r


def kernel(**inputs) -> np.ndarray:
    out, _ = _run(inputs, trace=False)
    return out


# revision 59
# speedup vs baseline: 2.3584x; 1.0922x over previous
"""Trainium2 Bass kernel for nn_BetweenClusterFC.

Computes out[e] = (emb_1[f[e]] @ W1 + b1) . (emb_2[t[e]] @ W2 + b2)
for E = 1.6M edges over N = 100k nodes, D_IN = 256, D_OUT = 128.

Strategy (8 NeuronCores, SPMD, full inputs in / full output out):
  - Nodes are split into 8 blocks of 12500.  Edges are assigned to cores by a
    (from-block-group, to-block-group) 4x2 rectangle: core c=(a,b) handles
    edges with from-node in blocks [4a..4a+3] and to-node in blocks
    [2b..2b+1].  Uniform (~200k edges/core); each core needs projections for
    4 from-blocks + 2 to-blocks (75k nodes).
  - All streamed data is bf16: the host pre-transposes and casts the
    embedding shards; the PE projects p = emb @ W (+ bias folded in as a
    K=1 ones x bias matmul), the Act engine moves psum -> sbuf in bf16 and
    issues the p-table writes (1KB-contiguous via a (p t)-permuted row
    order), keeping the DVE entirely free for the edge phase.
  - Edges are bucketed host-side by (local from-block, local to-block); per
    bucket both endpoint rows are fetched with SWDGE dma_gather calls (up to
    2048 idxs/call, two queues) using int16 permuted-local indices; a DVE
    bf16 multiply (2x mode) + f32 reduce produces the per-edge dots into a
    single SBUF-resident result tile, written out once at the end.
  - Bucket capacities are sized from the actual per-core edge counts (max
    across cores, 128-aligned), so the Bass program is built per problem
    instance (inspector-executor style) and cached.
  - The host applies the inverse edge permutation to assemble the output.

Written in raw Bass (explicit semaphores) - the Tile layer's generated sync
exceeds this toolchain's per-instruction wait-slot limits.
"""

import contextlib

import numpy as np

import concourse.bass as bass
import concourse.mybir as mybir

# ---------------------------------------------------------------- constants
N_NODES = 100_000
D_IN = 256
D_OUT = 128
N_EDGES = 1_600_000
N_CORES = 8

NB = 12_500          # nodes per block
NBP = 12_800         # padded block rows (25 * 512) - multiple of 512
NFB = 4              # from-blocks per core
NTB = 2              # to-blocks per core
NBUCKET = NFB * NTB  # 8 buckets per core

P1_ROWS = NFB * NBP  # 51200
P2_ROWS = NTB * NBP  # 25600

TILES1 = P1_ROWS // 128    # 400 node-tiles, table 1
TILES2 = P2_ROWS // 128    # 200 node-tiles, table 2
GROUPS1 = TILES1 // 4      # 100 psum groups
GROUPS2 = TILES2 // 4      # 50
NGROUP = GROUPS1 + GROUPS2  # 150
CHUNK_T = 20               # node-tiles per embT load chunk (= 5 groups)
NCH1 = TILES1 // CHUNK_T   # 20 chunks
NCH2 = TILES2 // CHUNK_T   # 10
NCHUNK = NCH1 + NCH2       # 30
EMB_COLS = CHUNK_T * 128   # 2560

MAX_CALL = 1024            # dma_gather idxs per call (HW limit; ring 4096/queue)

F32 = mybir.dt.float32
BF16 = mybir.dt.bfloat16
I16 = mybir.dt.int16
AX = mybir.AxisListType


# Projection block order: interleave p2/p1 blocks so gather buckets unblock
# as early as possible (bucket (fi,ti) needs p1 block fi + p2 block ti):
# (0,0)@50 groups, (0,1)@75, (1,*)@100, (2,*)@125, (3,*)@150.
BSEQ = [(1, 0), (0, 0), (1, 1), (0, 1), (0, 2), (0, 3)]
GPB = NBP // 512   # 25 groups per block
CPB = GPB // 5     # 5 chunks per block (CHUNK_T = 4 groups... 20 tiles)
GSEQ = []
CSEQ = []
for tab, blk in BSEQ:
    g0 = blk * GPB + (GROUPS1 if tab == 1 else 0)
    GSEQ += list(range(g0, g0 + GPB))
    c0 = blk * 5 + (NCH1 if tab == 1 else 0)
    CSEQ += list(range(c0, c0 + 5))
CPOS = {cid: q for q, cid in enumerate(CSEQ)}
_BPOS = {tb: i for i, tb in enumerate(BSEQ)}
# bucket (fi,ti) ready after this many groups in GSEQ order
BK_READY_Q = [GPB * (1 + max(_BPOS[(0, fi)], _BPOS[(1, ti)]))
              for fi in range(NFB) for ti in range(NTB)]


def _chunk_of_tile(tg):
    """global tile index -> (global chunk id, local col0 within chunk)."""
    if tg < TILES1:
        c = tg // CHUNK_T
        return c, (tg % CHUNK_T) * 128
    t2 = tg - TILES1
    c = NCH1 + t2 // CHUNK_T
    return c, (t2 % CHUNK_T) * 128


def _chunk_src(c):
    """global chunk id -> (table, col0)."""
    if c < NCH1:
        return 0, c * EMB_COLS
    return 1, (c - NCH1) * EMB_COLS


def _plan_calls(caps):
    """caps: per-bucket slot capacities (multiples of 128).
    Returns a list of gather-call PAIRS (bucket, n0, n1, idx_col0, slot_col0):
    two <=1024-idx gather calls whose outputs land adjacently so the DVE can
    process 2048 edges per op chain (n1 == 0 for a lone remainder call).
    Plus (idx_cols_total, slot_cols_total, per-bucket slot offsets)."""
    pairs = []
    icol = 0
    scol = 0
    boff = []
    for bk, cap in enumerate(caps):
        boff.append(scol)
        left = cap
        while left > 0:
            n0 = min(MAX_CALL, left)
            n1 = min(MAX_CALL, left - n0)
            pairs.append((bk, n0, n1, icol, scol))
            icol += (n0 + n1) // 16
            scol += (n0 + n1) // 128
            left -= n0 + n1
    return pairs, icol, scol, boff


# ---------------------------------------------------------------- device code
def build_bass(caps):
    """caps: per-bucket capacities (edge slots, multiples of 128), shared by
    all cores (max over cores)."""
    calls, idx_cols, tot_slots, boff = _plan_calls(caps)
    ncall = len(calls)

    nc = bass.Bass(dynamic_dma_scratch_size=65536, num_swdge_queues=2)

    e1t = nc.dram_tensor("e1t", [D_IN, P1_ROWS], BF16, kind="ExternalInput")
    e2t = nc.dram_tensor("e2t", [D_IN, P2_ROWS], BF16, kind="ExternalInput")
    w12 = nc.dram_tensor("w12", [D_IN, 2 * D_OUT], BF16, kind="ExternalInput")
    bo3 = nc.dram_tensor("bo3", [3, D_OUT], BF16, kind="ExternalInput")
    idxa = nc.dram_tensor("idxa", [128, idx_cols], I16, kind="ExternalInput")
    idxb = nc.dram_tensor("idxb", [128, idx_cols], I16, kind="ExternalInput")
    res = nc.dram_tensor("res", [128, tot_slots], F32, kind="ExternalOutput")

    p1d = nc.dram_tensor("p1d", [P1_ROWS, D_OUT], BF16, kind="Internal")
    p2d = nc.dram_tensor("p2d", [P2_ROWS, D_OUT], BF16, kind="Internal")
    pdst = (p1d, p2d)

    st = contextlib.ExitStack()
    with st:
        sb = lambda nm, shape, dt: st.enter_context(nc.sbuf_tensor(nm, shape, dt))
        sem = lambda nm: st.enter_context(nc.semaphore(name=nm))

        # wc columns: [w1_k0 | w2_k0 | w1_k1 | w2_k1] (128 each)
        wc = sb("wc", [128, 512], BF16)
        bt = [sb(f"bt{i}", [1, 128], BF16) for i in range(3)]  # b1, b2, ones
        idxt = (sb("idxta", [128, idx_cols], I16), sb("idxtb", [128, idx_cols], I16))
        et = [[sb(f"et_{p}_{k}", [128, EMB_COLS], BF16) for k in range(2)]
              for p in range(2)]  # [parity][k-half]
        pv = [sb(f"pv{i}", [128, 512], BF16) for i in range(4)]
        ps = [st.enter_context(nc.psum_tensor(f"ps{i}", [128, 512], F32))
              for i in range(4)]
        NBUF = 6  # gather-pair buffer rotation depth (5-stage DVE pipeline)
        at = [sb(f"at{i}", [128, 2 * MAX_CALL], BF16) for i in range(NBUF)]
        btg = [sb(f"btg{i}", [128, 2 * MAX_CALL], BF16) for i in range(NBUF)]
        rt = sb("rt", [128, tot_slots], F32)

        s_cl = sem("s_cl")               # w/b const loads (5 dmas -> 80)
        s_cli = sem("s_cli")             # idx table loads (2 dmas -> 32)
        s_load = (sem("s_load0"), sem("s_load1"))  # SP embT loads, by parity
        s_loadp = sem("s_loadp")         # Pool embT loads
        s_mm = sem("s_mm")               # matmuls (+1 each; 3 per tile)
        s_cp = sem("s_cp")               # act copies (+1 each)
        s_cpd = sem("s_cpd")             # dve copies (+1 each)
        s_pw = tuple(sem(f"s_pw{i}") for i in range(4))  # p-write dmas, by g%4
        # gather-completion sems by k%NBUF per queue: call k+NBUF's gathers are
        # gated on s_red >= k+1, so updates of each sem are causally ordered.
        s_ga = tuple(sem(f"s_ga{i}") for i in range(6))  # queue-0, +16/call
        s_gb = tuple(sem(f"s_gb{i}") for i in range(6))  # queue-1, +16/call
        s_st = tuple(sem(f"s_st{i}") for i in range(4))  # DVE dot stages 0-3
        s_stp = sem("s_stp")             # Pool-issued muls (stage 0)
        s_red = sem("s_red")             # final reduces (+1 per call)
        s_out = sem("s_out")             # res dma

        CONSTS = 5 * 16

        # per-sem cumulative gather counts: pair p incs s_g*[p%NBUF] by
        # 16 per issued call (2 for a full pair, 1 for a lone remainder)
        _sem_cnt = [0] * NBUF
        gwait = []
        for _p, (_bk, _n0, _n1, _i, _s) in enumerate(calls):
            _sem_cnt[_p % NBUF] += 16 * (2 if _n1 else 1)
            gwait.append(_sem_cnt[_p % NBUF])

        # pairs whose stage-0 multiply runs on the Pool engine.  Disabled: the
        # Pool must keep the mlp ucode library loaded for dma_gather, which
        # excludes the standard library's TensorTensor.
        pool_mul = [False] * ncall

        # During the pre-gather phase the DVE is mostly idle: it takes the odd
        # groups' psum->pv copies for the first PH1 groups (through the
        # bucket-1 gate at 75 so that gate opens before the DVE needs it).
        PH1 = 48
        dve_copy = [q < PH1 and q % 2 == 1 for q in range(NGROUP)]
        cp_cnt = []  # cumulative (act, dve) copy counts through group q
        _na = _ndv = 0
        for q in range(NGROUP):
            if dve_copy[q]:
                _ndv += 1
            else:
                _na += 1
            cp_cnt.append((_na, _ndv))

        def _copy_wait(eng, q):
            """wait until the copy of group q is complete."""
            if dve_copy[q]:
                eng.wait_ge(s_cpd, cp_cnt[q][1])
            else:
                eng.wait_ge(s_cp, cp_cnt[q][0])
        mul_cnt = []
        _nd = _np_ = 0
        for p in range(ncall):
            if pool_mul[p]:
                _np_ += 1
            else:
                _nd += 1
            mul_cnt.append((_nd, _np_))

        block = st.enter_context(nc.Block())

        # Chunks 1,3,5,7,9 are loaded by the Pool engine during its pre-gather
        # idle window (own sem s_loadp); SP loads the rest (s_load by parity).
        POOL_CHUNKS = ()
        chunk_wait = {}
        _cnt = {0: 0, 1: 0, 'p': 0}
        for _cq in range(NCHUNK):
            key = 'p' if _cq in POOL_CHUNKS else _cq % 2
            _cnt[key] += 32
            chunk_wait[_cq] = (key, _cnt[key])

        # p-writes: all by Act (SP/Pool splits measured slower)
        def write_eng_is_sp(q):
            return False

        def p_write(eng, q):
            g = GSEQ[q]
            tab = 0 if g < GROUPS1 else 1
            r0 = g * 512 if tab == 0 else (g - GROUPS1) * 512
            # (p t)-permuted write: table row r0 + p*4 + t <- node r0 + t*128 + p
            eng.dma_start(
                out=pdst[tab][r0:r0 + 512, :].rearrange("(p t) d -> p t d", p=128),
                in_=pv[q % 4][:].rearrange("p (t d) -> p t d", d=128),
            ).then_inc(s_pw[q % 4], 16)

        def load_chunk(eng, cq):
            if cq >= 2:
                # buffer cq%2 previously held chunk cq-2; wait consumed
                eng.wait_ge(s_mm, 3 * CHUNK_T * (cq - 1))
            tab, col0 = _chunk_src(CSEQ[cq])
            src = e1t if tab == 0 else e2t
            par = cq % 2
            s_ld = s_loadp if cq in POOL_CHUNKS else s_load[par]
            eng.dma_start(out=et[par][0][:],
                          in_=src[0:128, col0:col0 + EMB_COLS]).then_inc(s_ld, 16)
            eng.dma_start(out=et[par][1][:],
                          in_=src[128:256, col0:col0 + EMB_COLS]).then_inc(s_ld, 16)

        # ------------------------------------------------ SP: embT + p-writes
        @block.sync
        def _(sync):
            for k in range(2):
                sync.dma_start(out=wc[:, k * 256:(k + 1) * 256],
                               in_=w12[k * 128:(k + 1) * 128, :]).then_inc(s_cl, 16)
            for i in range(3):
                sync.dma_start(out=bt[i][:], in_=bo3[i:i + 1, :]).then_inc(s_cl, 16)
            load_chunk(sync, 0)
            load_chunk(sync, 1)
            next_cq = 2
            for q in range(NGROUP):
                # look ahead: issue loads for chunks starting within 5 groups
                while next_cq < NCHUNK and next_cq * CHUNK_T <= (q + 5) * 4 + 3:
                    if next_cq not in POOL_CHUNKS:
                        load_chunk(sync, next_cq)
                    next_cq += 1
                if write_eng_is_sp(q):
                    _copy_wait(sync, q)
                    p_write(sync, q)

        # ------------------------------------------------ PE: projections
        @block.tensor
        def _(tensor):
            tensor.wait_ge(s_cl, CONSTS)
            for q, g in enumerate(GSEQ):
                tab = 0 if g < GROUPS1 else 1
                bc = bt[tab][:]
                for j in range(4):
                    tq = q * 4 + j
                    cid, col0 = _chunk_of_tile(g * 4 + j)
                    cq = CPOS[cid]
                    if tq == cq * CHUNK_T:  # first processed tile of chunk
                        key, cntv = chunk_wait[cq]
                        tensor.wait_ge(s_loadp if key == 'p' else s_load[key], cntv)
                    if j == 0 and q >= 4:
                        _copy_wait(tensor, q - 4)  # psum bank q%4 free
                    out = ps[q % 4][:, j * 128:(j + 1) * 128]
                    tensor.matmul(out=out, lhsT=et[cq % 2][0][:, col0:col0 + 128],
                                  rhs=wc[:, tab * 128:tab * 128 + 128],
                                  start=True, stop=False).then_inc(s_mm, 1)
                    tensor.matmul(out=out, lhsT=et[cq % 2][1][:, col0:col0 + 128],
                                  rhs=wc[:, 256 + tab * 128:256 + tab * 128 + 128],
                                  start=False, stop=False).then_inc(s_mm, 1)
                    tensor.matmul(out=out, lhsT=bt[2][:], rhs=bc,
                                  start=False, stop=True).then_inc(s_mm, 1)

        # ------------------------------------------------ Act: psum->sbuf + DMAs
        @block.scalar
        def _(scalar):
            scalar.wait_ge(s_cl, CONSTS)
            # p-write of group q-1 is issued after the copy of group q so the
            # copy-completion sem is already visible when its write issues;
            # groups >= SPW are written by SP instead.
            for q, g in enumerate(GSEQ):
                if not dve_copy[q]:
                    scalar.wait_ge(s_mm, 12 * q + 12)
                    if q >= 4:
                        scalar.wait_ge(s_pw[q % 4], 16 * (q // 4))  # pv drained
                    scalar.copy(out=pv[q % 4][:], in_=ps[q % 4][:]).then_inc(s_cp, 1)
                if q >= 1 and not write_eng_is_sp(q - 1):
                    _copy_wait(scalar, q - 1)
                    p_write(scalar, q - 1)
            _copy_wait(scalar, NGROUP - 1)
            p_write(scalar, NGROUP - 1)
            # write results in two halves so only the last sliver is exposed
            half_p = ncall // 2
            half_s = calls[half_p][4]
            scalar.wait_ge(s_red, half_p)
            scalar.dma_start(out=res[:, :half_s], in_=rt[:, :half_s]).then_inc(s_out, 16)
            scalar.wait_ge(s_red, ncall)
            scalar.dma_start(out=res[:, half_s:], in_=rt[:, half_s:]).then_inc(s_out, 16)
            scalar.wait_ge(s_out, 32)

        # ------------------------------------------------ Pool: gathers
        @block.gpsimd
        def _(gpsimd):
            # the Pool engine is idle until the first bucket gate (~40us): it
            # loads the consts, every other early embT chunk, and its own idx
            # tables, so SP's chunk stream (which feeds the PE) never blocks.
            gpsimd.dma_start(out=idxt[0][:], in_=idxa[:]).then_inc(s_cli, 16)
            gpsimd.dma_start(out=idxt[1][:], in_=idxb[:]).then_inc(s_cli, 16)
            from concourse import library_config
            gpsimd.load_library(library_config.mlp)
            sizes = ({c[1] for c in calls} | {c[2] for c in calls}) - {0}
            regs = {n: gpsimd.to_reg(n) for n in sorted(sizes)}
            gpsimd.wait_ge(s_cl, CONSTS)
            gpsimd.wait_ge(s_cli, 32)

            def pool_stage0(p):
                bk, n0, n1, icol, scol = calls[p]
                S = (n0 + n1) // 128
                gpsimd.wait_ge(s_ga[p % NBUF], gwait[p])
                gpsimd.wait_ge(s_gb[p % NBUF], gwait[p])
                a3 = at[p % NBUF][:, :S * 128]
                gpsimd.tensor_mul(out=a3, in0=a3,
                                  in1=btg[p % NBUF][:, :S * 128]).then_inc(s_stp, 1)

            gated_bk = -1
            for p, (bk, n0, n1, icol, scol) in enumerate(calls):
                fi, ti = bk // NTB, bk % NTB
                if bk > gated_bk:
                    gated_bk = bk
                    nq = BK_READY_Q[bk]
                    for r in range(4):
                        gpsimd.wait_ge(s_pw[r], 16 * len(range(r, nq, 4)))
                if p >= NBUF:
                    gpsimd.wait_ge(s_red, p - NBUF + 1)  # at/btg[p%NBUF] consumed
                if p >= 2 and pool_mul[p - 2]:
                    pool_stage0(p - 2)
                # rows are moved as 64 packed f32 (= 128 bf16): same bytes and
                # descriptors, half the modeled element count.
                nh = 0
                for h, n in enumerate((n0, n1)):
                    if n == 0:
                        continue
                    S = n // 128
                    c0 = icol + nh // 16
                    gpsimd.dma_gather(
                        out_ap=at[p % NBUF][:, nh:nh + S * 128].bitcast(F32)
                            .rearrange("p (s d) -> p s d", d=64),
                        in_ap=p1d[fi * NBP:(fi + 1) * NBP, :].bitcast(F32),
                        idxs_ap=idxt[0][:, c0:c0 + n // 16],
                        num_idxs=n, num_idxs_reg=regs[n], elem_size=D_OUT // 2,
                        queue_num=0,
                    ).then_inc(s_ga[p % NBUF], 16)
                    gpsimd.dma_gather(
                        out_ap=btg[p % NBUF][:, nh:nh + S * 128].bitcast(F32)
                            .rearrange("p (s d) -> p s d", d=64),
                        in_ap=p2d[ti * NBP:(ti + 1) * NBP, :].bitcast(F32),
                        idxs_ap=idxt[1][:, c0:c0 + n // 16],
                        num_idxs=n, num_idxs_reg=regs[n], elem_size=D_OUT // 2,
                        queue_num=1,
                    ).then_inc(s_gb[p % NBUF], 16)
                    nh += S * 128
            for p in range(max(0, ncall - 2), ncall):
                if pool_mul[p]:
                    pool_stage0(p)

        # ------------------------------------------------ DVE: dot products
        # bf16 multiply (2x mode), tree-halve in bf16 (2x) down to 16 partials
        # per slot, then one short f32 reduce - one chain per gather pair
        # (up to 2048 edges).  The 5-stage chain is software-pipelined across
        # pairs (stage s of pair p in round p+s) so every intra-chain
        # semaphore is already visible when its wait issues.
        @block.vector
        def _(vector):
            # phase-1 assist: odd-group psum->pv copies while gathers are gated
            for q in range(NGROUP):
                if not dve_copy[q]:
                    continue
                vector.wait_ge(s_mm, 12 * q + 12)
                if q >= 4:
                    vector.wait_ge(s_pw[q % 4], 16 * (q // 4))  # pv drained
                vector.tensor_copy(out=pv[q % 4][:], in_=ps[q % 4][:]).then_inc(s_cpd, 1)

            def stage(s, p):
                bk, n0, n1, icol, scol = calls[p]
                S = (n0 + n1) // 128
                av = at[p % NBUF][:, :S * 128].rearrange("p (s d) -> p s d", d=128)
                if s == 0:
                    if pool_mul[p]:
                        return  # multiply issued by the Pool engine
                    vector.wait_ge(s_ga[p % NBUF], gwait[p])
                    vector.wait_ge(s_gb[p % NBUF], gwait[p])
                    a3 = at[p % NBUF][:, :S * 128]
                    vector.tensor_mul(out=a3, in0=a3,
                                      in1=btg[p % NBUF][:, :S * 128]).then_inc(s_st[0], 1)
                elif s == 1:
                    if pool_mul[p]:
                        vector.wait_ge(s_stp, mul_cnt[p][1])
                    else:
                        vector.wait_ge(s_st[0], mul_cnt[p][0])
                    vector.tensor_add(out=av[:, :, 0:64], in0=av[:, :, 0:64],
                                      in1=av[:, :, 64:128]).then_inc(s_st[1], 1)
                elif s in (2, 3):
                    h = 128 >> s  # 32, 16
                    vector.wait_ge(s_st[s - 1], p + 1)
                    vector.tensor_add(out=av[:, :, 0:h], in0=av[:, :, 0:h],
                                      in1=av[:, :, h:2 * h]).then_inc(s_st[s], 1)
                else:
                    vector.wait_ge(s_st[3], p + 1)
                    vector.reduce_sum(out=rt[:, scol:scol + S],
                                      in_=av[:, :, 0:16], axis=AX.X).then_inc(s_red, 1)

            for r in range(ncall + 4):
                for s in range(4, -1, -1):
                    p = r - s
                    if 0 <= p < ncall:
                        stage(s, p)

    return nc, calls, boff


_NC_CACHE: dict = {}


def _get_nc(caps):
    key = tuple(caps)
    if key not in _NC_CACHE:
        nc, calls, boff = build_bass(caps)
        from concourse.library_overlay import lower_extended_insts
        lower_extended_insts(nc)
        _NC_CACHE[key] = (nc, calls, boff)
    return _NC_CACHE[key]


# ---------------------------------------------------------------- host side
def _perm_local(n):
    """block-local node id -> permuted table row (within block).
    Table row g*512 + p*4 + t holds node g*512 + t*128 + p."""
    g, u = np.divmod(n, 512)
    t, p = np.divmod(u, 128)
    return g * 512 + p * 4 + t


def _marshal(emb_1, emb_2, nodes_from_to, W1, b1, W2, b2):
    """Shard/bucket inputs per core.  Returns (caps, in_maps, books)."""
    import ml_dtypes
    bf16 = ml_dtypes.bfloat16

    f = np.asarray(nodes_from_to[:, 0], dtype=np.int64)
    t = np.asarray(nodes_from_to[:, 1], dtype=np.int64)
    emb_1 = np.asarray(emb_1, dtype=np.float32)
    emb_2 = np.asarray(emb_2, dtype=np.float32)
    w12 = np.concatenate(
        [np.asarray(W1, dtype=np.float32), np.asarray(W2, dtype=np.float32)],
        axis=1).astype(bf16)
    bo3 = np.stack([
        np.asarray(b1, dtype=np.float32).reshape(-1),
        np.asarray(b2, dtype=np.float32).reshape(-1),
        np.ones(D_OUT, np.float32),
    ]).astype(bf16)

    core = (f // (NFB * NB)) * 4 + t // (NTB * NB)
    order0 = np.argsort(core, kind="stable")
    ccnt = np.bincount(core, minlength=N_CORES)
    coff = np.concatenate([[0], np.cumsum(ccnt)])

    percore = []
    all_cnts = np.zeros((N_CORES, NBUCKET), np.int64)
    for c in range(N_CORES):
        a, b = c // 4, c % 4
        sel = order0[coff[c]:coff[c + 1]]
        fc, tcv = f[sel], t[sel]
        fi = fc // NB - NFB * a
        ti = tcv // NB - NTB * b
        fl = _perm_local(fc % NB).astype(np.int16)
        tl = _perm_local(tcv % NB).astype(np.int16)
        bk = fi * NTB + ti
        o2 = np.argsort(bk, kind="stable")
        sel2, fl2, tl2 = sel[o2], fl[o2], tl[o2]
        cnts = np.bincount(bk, minlength=NBUCKET)
        all_cnts[c] = cnts
        percore.append((a, b, sel2, fl2, tl2, cnts))

    caps = [int(-(-all_cnts[:, k].max() // 128) * 128) for k in range(NBUCKET)]
    calls, idx_cols, tot_slots, boff = _plan_calls(caps)

    in_maps, books = [], []
    for c in range(N_CORES):
        a, b, sel2, fl2, tl2, cnts = percore[c]
        pos = np.concatenate([[0], np.cumsum(cnts)])

        slots_a = np.zeros((NBUCKET, max(caps)), np.int16)
        slots_b = np.zeros((NBUCKET, max(caps)), np.int16)
        for k in range(NBUCKET):
            slots_a[k, :cnts[k]] = fl2[pos[k]:pos[k + 1]]
            slots_b[k, :cnts[k]] = tl2[pos[k]:pos[k + 1]]
        # wrap by 16: idx i of a bucket at (partition i%16, col i//16),
        # replicated across the 8 groups of 16 partitions
        wa_cols = []
        wb_cols = []
        for k in range(NBUCKET):
            cap = caps[k]
            wa_cols.append(slots_a[k, :cap].reshape(cap // 16, 16).T)
            wb_cols.append(slots_b[k, :cap].reshape(cap // 16, 16).T)
        idxa = np.tile(np.concatenate(wa_cols, axis=1), (8, 1))
        idxb = np.tile(np.concatenate(wb_cols, axis=1), (8, 1))

        e1t = np.zeros((D_IN, P1_ROWS), bf16)
        for i in range(NFB):
            blk = emb_1[(NFB * a + i) * NB:(NFB * a + i + 1) * NB]
            e1t[:, i * NBP:i * NBP + NB] = blk.T.astype(bf16)
        e2t = np.zeros((D_IN, P2_ROWS), bf16)
        for i in range(NTB):
            blk = emb_2[(NTB * b + i) * NB:(NTB * b + i + 1) * NB]
            e2t[:, i * NBP:i * NBP + NB] = blk.T.astype(bf16)

        in_maps.append({
            "e1t": e1t, "e2t": e2t, "w12": w12, "bo3": bo3,
            "idxa": np.ascontiguousarray(idxa),
            "idxb": np.ascontiguousarray(idxb),
        })
        books.append((sel2, cnts, pos))
    return caps, in_maps, books


def _unmarshal(results, books, caps, n_edges):
    calls, idx_cols, tot_slots, boff = _plan_calls(caps)
    out = np.empty(n_edges, np.float32)
    for c in range(N_CORES):
        sel2, cnts, pos = books[c]
        r = results[c]["res"]  # [128, tot_slots]
        for k in range(NBUCKET):
            if cnts[k] == 0:
                continue
            s0 = boff[k]
            nslots = caps[k] // 128
            stream = r[:, s0:s0 + nslots].T.reshape(-1)
            out[sel2[pos[k]:pos[k + 1]]] = stream[:cnts[k]]
    return out


def _run(inputs, trace=False, **run_kwargs):
    from concourse.bass_utils import run_bass_kernel_spmd

    caps, in_maps, books = _marshal(**inputs)
    nc, calls, boff = _get_nc(caps)
    r = run_bass_kernel_spmd(
        nc, in_maps, core_ids=list(range(N_CORES)), trace=trace, **run_kwargs
    )
    out = _unmarshal(r.results, books, caps, len(inputs["nodes_from_to"]))
    return out, r


def kernel(**inputs) -> np.ndarray:
    out, _ = _run(inputs, trace=False)
    return out


# revision 68
# speedup vs baseline: 2.3958x; 1.0159x over previous
"""Trainium2 Bass kernel for nn_BetweenClusterFC.

Computes out[e] = (emb_1[f[e]] @ W1 + b1) . (emb_2[t[e]] @ W2 + b2)
for E = 1.6M edges over N = 100k nodes, D_IN = 256, D_OUT = 128.

Strategy (8 NeuronCores, SPMD, full inputs in / full output out):
  - Nodes are split into 8 blocks of 12500.  Edges are assigned to cores by a
    (from-block-group, to-block-group) 4x2 rectangle: core c=(a,b) handles
    edges with from-node in blocks [4a..4a+3] and to-node in blocks
    [2b..2b+1].  Uniform (~200k edges/core); each core needs projections for
    4 from-blocks + 2 to-blocks (75k nodes).
  - All streamed data is bf16: the host pre-transposes and casts the
    embedding shards; the PE projects p = emb @ W (+ bias folded in as a
    K=1 ones x bias matmul), the Act engine moves psum -> sbuf in bf16 and
    issues the p-table writes (1KB-contiguous via a (p t)-permuted row
    order), keeping the DVE entirely free for the edge phase.
  - Edges are bucketed host-side by (local from-block, local to-block); per
    bucket both endpoint rows are fetched with SWDGE dma_gather calls (1024
    idxs/call HW limit, paired so the DVE sees 2048-edge slabs, two DMA
    queues, rows moved as packed f32 pairs) using int16 permuted-local
    indices; the DVE runs a software-pipelined bf16 multiply (2x mode) +
    bf16 tree-halving + short f32 reduce per slab into a single
    SBUF-resident result tile, written out in two halves.
  - Bucket capacities are sized from the actual per-core edge counts (max
    across cores, 128-aligned), so the Bass program is built per problem
    instance (inspector-executor style) and cached.  Bucket 0 is ordered
    early-prefix-rows first so its first gather pairs unblock after 37 of
    150 projection groups; the idle pre-gather Pool engine prefetches the
    idx tables and part of the embT stream; the DVE covers half the early
    psum->sbuf copies.
  - The host applies the inverse edge permutation to assemble the output.

Written in raw Bass (explicit semaphores) - the Tile layer's generated sync
exceeds this toolchain's per-instruction wait-slot limits.
"""

import contextlib

import numpy as np

import concourse.bass as bass
import concourse.mybir as mybir

# ---------------------------------------------------------------- constants
N_NODES = 100_000
D_IN = 256
D_OUT = 128
N_EDGES = 1_600_000
N_CORES = 8

NB = 12_500          # nodes per block
NBP = 12_800         # padded block rows (25 * 512) - multiple of 512
NFB = 4              # from-blocks per core
NTB = 2              # to-blocks per core
NBUCKET = NFB * NTB  # 8 buckets per core

P1_ROWS = NFB * NBP  # 51200
P2_ROWS = NTB * NBP  # 25600

TILES1 = P1_ROWS // 128    # 400 node-tiles, table 1
TILES2 = P2_ROWS // 128    # 200 node-tiles, table 2
GROUPS1 = TILES1 // 4      # 100 psum groups
GROUPS2 = TILES2 // 4      # 50
NGROUP = GROUPS1 + GROUPS2  # 150
CHUNK_T = 20               # node-tiles per embT load chunk (= 5 groups)
NCH1 = TILES1 // CHUNK_T   # 20 chunks
NCH2 = TILES2 // CHUNK_T   # 10
NCHUNK = NCH1 + NCH2       # 30
EMB_COLS = CHUNK_T * 128   # 2560

MAX_CALL = 1024            # dma_gather idxs per call (HW limit; ring 4096/queue)
HROWS = 6_144              # p1-block-0 prefix rows for early bucket-0 pairs
PH1 = 48                   # DVE assists psum->pv copies for odd groups < PH1

F32 = mybir.dt.float32
BF16 = mybir.dt.bfloat16
I16 = mybir.dt.int16
AX = mybir.AxisListType


# Projection block order: interleave p2/p1 blocks so gather buckets unblock
# as early as possible (bucket (fi,ti) needs p1 block fi + p2 block ti):
# (0,0)@50 groups, (0,1)@75, (1,*)@100, (2,*)@125, (3,*)@150.
BSEQ = [(1, 0), (0, 0), (1, 1), (0, 1), (0, 2), (0, 3)]
GPB = NBP // 512   # 25 groups per block
CPB = GPB // 5     # 5 chunks per block (CHUNK_T = 4 groups... 20 tiles)
GSEQ = []
CSEQ = []
for tab, blk in BSEQ:
    g0 = blk * GPB + (GROUPS1 if tab == 1 else 0)
    GSEQ += list(range(g0, g0 + GPB))
    c0 = blk * 5 + (NCH1 if tab == 1 else 0)
    CSEQ += list(range(c0, c0 + 5))
CPOS = {cid: q for q, cid in enumerate(CSEQ)}
_BPOS = {tb: i for i, tb in enumerate(BSEQ)}
# bucket (fi,ti) ready after this many groups in GSEQ order
BK_READY_Q = [GPB * (1 + max(_BPOS[(0, fi)], _BPOS[(1, ti)]))
              for fi in range(NFB) for ti in range(NTB)]


def _chunk_of_tile(tg):
    """global tile index -> (global chunk id, local col0 within chunk)."""
    if tg < TILES1:
        c = tg // CHUNK_T
        return c, (tg % CHUNK_T) * 128
    t2 = tg - TILES1
    c = NCH1 + t2 // CHUNK_T
    return c, (t2 % CHUNK_T) * 128


def _chunk_src(c):
    """global chunk id -> (table, col0)."""
    if c < NCH1:
        return 0, c * EMB_COLS
    return 1, (c - NCH1) * EMB_COLS


def _plan_calls(caps):
    """caps: per-bucket slot capacities (multiples of 128).
    Returns a list of gather-call PAIRS (bucket, n0, n1, idx_col0, slot_col0):
    two <=1024-idx gather calls whose outputs land adjacently so the DVE can
    process 2048 edges per op chain (n1 == 0 for a lone remainder call).
    Plus (idx_cols_total, slot_cols_total, per-bucket slot offsets)."""
    pairs = []
    icol = 0
    scol = 0
    boff = []
    for bk, cap in enumerate(caps):
        boff.append(scol)
        left = cap
        while left > 0:
            n0 = min(MAX_CALL, left)
            n1 = min(MAX_CALL, left - n0)
            pairs.append((bk, n0, n1, icol, scol))
            icol += (n0 + n1) // 16
            scol += (n0 + n1) // 128
            left -= n0 + n1
    return pairs, icol, scol, boff


# ---------------------------------------------------------------- device code
def build_bass(caps, ep0):
    """caps: per-bucket capacities (edge slots, multiples of 128), shared by
    all cores (max over cores).  ep0: bucket-0 pairs gathering only the
    HROWS-row prefix of p1 block 0 (host orders those edges first)."""
    calls, idx_cols, tot_slots, boff = _plan_calls(caps)
    ncall = len(calls)

    nc = bass.Bass(dynamic_dma_scratch_size=65536, num_swdge_queues=2)

    e1t = nc.dram_tensor("e1t", [D_IN, P1_ROWS], BF16, kind="ExternalInput")
    e2t = nc.dram_tensor("e2t", [D_IN, P2_ROWS], BF16, kind="ExternalInput")
    w12 = nc.dram_tensor("w12", [D_IN, 2 * D_OUT], BF16, kind="ExternalInput")
    bo3 = nc.dram_tensor("bo3", [3, D_OUT], BF16, kind="ExternalInput")
    idxa = nc.dram_tensor("idxa", [128, idx_cols], I16, kind="ExternalInput")
    idxb = nc.dram_tensor("idxb", [128, idx_cols], I16, kind="ExternalInput")
    res = nc.dram_tensor("res", [128, tot_slots], F32, kind="ExternalOutput")

    p1d = nc.dram_tensor("p1d", [P1_ROWS, D_OUT], BF16, kind="Internal")
    p2d = nc.dram_tensor("p2d", [P2_ROWS, D_OUT], BF16, kind="Internal")
    pdst = (p1d, p2d)

    st = contextlib.ExitStack()
    with st:
        sb = lambda nm, shape, dt: st.enter_context(nc.sbuf_tensor(nm, shape, dt))
        sem = lambda nm: st.enter_context(nc.semaphore(name=nm))

        # wc columns: [w1_k0 | w2_k0 | w1_k1 | w2_k1] (128 each)
        wc = sb("wc", [128, 512], BF16)
        bt = [sb(f"bt{i}", [1, 128], BF16) for i in range(3)]  # b1, b2, ones
        idxt = (sb("idxta", [128, idx_cols], I16), sb("idxtb", [128, idx_cols], I16))
        et = [[sb(f"et_{p}_{k}", [128, EMB_COLS], BF16) for k in range(2)]
              for p in range(3)]  # [buffer cq%3][k-half]
        pv = [sb(f"pv{i}", [128, 512], BF16) for i in range(4)]
        ps = [st.enter_context(nc.psum_tensor(f"ps{i}", [128, 512], F32))
              for i in range(4)]
        NBUF = 6  # gather-pair buffer rotation depth (5-stage DVE pipeline)
        at = [sb(f"at{i}", [128, 2 * MAX_CALL], BF16) for i in range(NBUF)]
        btg = [sb(f"btg{i}", [128, 2 * MAX_CALL], BF16) for i in range(NBUF)]
        rt = sb("rt", [128, tot_slots], F32)

        s_cl = sem("s_cl")               # w/b const loads (5 dmas -> 80)
        s_cli = sem("s_cli")             # idx table loads (2 dmas -> 32)
        s_load = tuple(sem(f"s_load{i}") for i in range(3))  # SP embT, by cq%3
        s_loadp = sem("s_loadp")         # Pool embT loads
        s_mm = sem("s_mm")               # matmuls (+1 each; 3 per tile)
        s_cp = sem("s_cp")               # act copies (+1 each)
        s_cpd = sem("s_cpd")             # dve copies (+1 each)
        s_pw = tuple(sem(f"s_pw{i}") for i in range(4))  # p-write dmas, by g%4
        # gather-completion sems by k%NBUF per queue: call k+NBUF's gathers are
        # gated on s_red >= k+1, so updates of each sem are causally ordered.
        s_ga = tuple(sem(f"s_ga{i}") for i in range(6))  # queue-0, +16/call
        s_gb = tuple(sem(f"s_gb{i}") for i in range(6))  # queue-1, +16/call
        s_st = tuple(sem(f"s_st{i}") for i in range(4))  # DVE dot stages 0-3
        s_stp = sem("s_stp")             # Pool-issued muls (stage 0)
        s_red = sem("s_red")             # final reduces (+1 per call)
        s_out = sem("s_out")             # res dma

        CONSTS = 5 * 16

        # per-sem cumulative gather counts: pair p incs s_g*[p%NBUF] by
        # 16 per issued call (2 for a full pair, 1 for a lone remainder)
        _sem_cnt = [0] * NBUF
        gwait = []
        for _p, (_bk, _n0, _n1, _i, _s) in enumerate(calls):
            _sem_cnt[_p % NBUF] += 16 * (2 if _n1 else 1)
            gwait.append(_sem_cnt[_p % NBUF])

        # pairs whose stage-0 multiply runs on the Pool engine.  Disabled: the
        # Pool must keep the mlp ucode library loaded for dma_gather, which
        # excludes the standard library's TensorTensor.
        pool_mul = [False] * ncall

        # During the pre-gather phase the DVE is mostly idle: it takes the odd
        # groups' psum->pv copies for the first PH1 groups.
        dve_copy = [q < PH1 and q % 2 == 1 for q in range(NGROUP)]
        cp_cnt = []  # cumulative (act, dve) copy counts through group q
        _na = _ndv = 0
        for q in range(NGROUP):
            if dve_copy[q]:
                _ndv += 1
            else:
                _na += 1
            cp_cnt.append((_na, _ndv))

        def _copy_wait(eng, q):
            """wait until the copy of group q is complete."""
            if dve_copy[q]:
                eng.wait_ge(s_cpd, cp_cnt[q][1])
            else:
                eng.wait_ge(s_cp, cp_cnt[q][0])
        mul_cnt = []
        _nd = _np_ = 0
        for p in range(ncall):
            if pool_mul[p]:
                _np_ += 1
            else:
                _nd += 1
            mul_cnt.append((_nd, _np_))

        block = st.enter_context(nc.Block())

        # Chunks 1,3,5,7,9 are loaded by the Pool engine during its pre-gather
        # idle window (own sem s_loadp); SP loads the rest (s_load by parity).
        POOL_CHUNKS = (4, 6, 8)
        chunk_wait = {}
        _cnt = {0: 0, 1: 0, 2: 0, 'p': 0}
        for _cq in range(NCHUNK):
            key = 'p' if _cq in POOL_CHUNKS else _cq % 3
            _cnt[key] += 32
            chunk_wait[_cq] = (key, _cnt[key])

        # p-writes: all by Act (SP/Pool splits measured slower)
        def write_eng_is_sp(q):
            return False

        def p_write(eng, q):
            g = GSEQ[q]
            tab = 0 if g < GROUPS1 else 1
            r0 = g * 512 if tab == 0 else (g - GROUPS1) * 512
            # (p t)-permuted write: table row r0 + p*4 + t <- node r0 + t*128 + p
            eng.dma_start(
                out=pdst[tab][r0:r0 + 512, :].rearrange("(p t) d -> p t d", p=128),
                in_=pv[q % 4][:].rearrange("p (t d) -> p t d", d=128),
            ).then_inc(s_pw[q % 4], 16)

        def load_chunk(eng, cq):
            if cq >= 3:
                # buffer cq%3 previously held chunk cq-3; wait consumed
                eng.wait_ge(s_mm, 3 * CHUNK_T * (cq - 2))
            tab, col0 = _chunk_src(CSEQ[cq])
            src = e1t if tab == 0 else e2t
            par = cq % 3
            s_ld = s_loadp if cq in POOL_CHUNKS else s_load[cq % 3]
            eng.dma_start(out=et[par][0][:],
                          in_=src[0:128, col0:col0 + EMB_COLS]).then_inc(s_ld, 16)
            eng.dma_start(out=et[par][1][:],
                          in_=src[128:256, col0:col0 + EMB_COLS]).then_inc(s_ld, 16)

        # ------------------------------------------------ SP: embT + p-writes
        @block.sync
        def _(sync):
            for k in range(2):
                sync.dma_start(out=wc[:, k * 256:(k + 1) * 256],
                               in_=w12[k * 128:(k + 1) * 128, :]).then_inc(s_cl, 16)
            for i in range(3):
                sync.dma_start(out=bt[i][:], in_=bo3[i:i + 1, :]).then_inc(s_cl, 16)
            load_chunk(sync, 0)
            load_chunk(sync, 1)
            next_cq = 2
            for q in range(NGROUP):
                # look ahead: issue loads for chunks starting within 5 groups
                while next_cq < NCHUNK and next_cq * CHUNK_T <= (q + 5) * 4 + 3:
                    if next_cq not in POOL_CHUNKS:
                        load_chunk(sync, next_cq)
                    next_cq += 1
                if write_eng_is_sp(q):
                    _copy_wait(sync, q)
                    p_write(sync, q)

        # ------------------------------------------------ PE: projections
        @block.tensor
        def _(tensor):
            tensor.wait_ge(s_cl, CONSTS)
            for q, g in enumerate(GSEQ):
                tab = 0 if g < GROUPS1 else 1
                bc = bt[tab][:]
                for j in range(4):
                    tq = q * 4 + j
                    cid, col0 = _chunk_of_tile(g * 4 + j)
                    cq = CPOS[cid]
                    if tq == cq * CHUNK_T:  # first processed tile of chunk
                        key, cntv = chunk_wait[cq]
                        tensor.wait_ge(s_loadp if key == 'p' else s_load[key], cntv)
                    if j == 0 and q >= 4:
                        _copy_wait(tensor, q - 4)  # psum bank q%4 free
                    out = ps[q % 4][:, j * 128:(j + 1) * 128]
                    tensor.matmul(out=out, lhsT=et[cq % 3][0][:, col0:col0 + 128],
                                  rhs=wc[:, tab * 128:tab * 128 + 128],
                                  start=True, stop=False).then_inc(s_mm, 1)
                    tensor.matmul(out=out, lhsT=et[cq % 3][1][:, col0:col0 + 128],
                                  rhs=wc[:, 256 + tab * 128:256 + tab * 128 + 128],
                                  start=False, stop=False).then_inc(s_mm, 1)
                    tensor.matmul(out=out, lhsT=bt[2][:], rhs=bc,
                                  start=False, stop=True).then_inc(s_mm, 1)

        # ------------------------------------------------ Act: psum->sbuf + DMAs
        @block.scalar
        def _(scalar):
            scalar.wait_ge(s_cl, CONSTS)
            # p-write of group q-1 is issued after the copy of group q so the
            # copy-completion sem is already visible when its write issues;
            # groups >= SPW are written by SP instead.
            for q, g in enumerate(GSEQ):
                if not dve_copy[q]:
                    scalar.wait_ge(s_mm, 12 * q + 12)
                    if q >= 4:
                        scalar.wait_ge(s_pw[q % 4], 16 * (q // 4))  # pv drained
                    scalar.copy(out=pv[q % 4][:], in_=ps[q % 4][:]).then_inc(s_cp, 1)
                if q >= 1 and not write_eng_is_sp(q - 1):
                    _copy_wait(scalar, q - 1)
                    p_write(scalar, q - 1)
            _copy_wait(scalar, NGROUP - 1)
            p_write(scalar, NGROUP - 1)
            # write results in two halves so only the last sliver is exposed
            half_p = ncall // 2
            half_s = calls[half_p][4]
            scalar.wait_ge(s_red, half_p)
            scalar.dma_start(out=res[:, :half_s], in_=rt[:, :half_s]).then_inc(s_out, 16)
            scalar.wait_ge(s_red, ncall)
            scalar.dma_start(out=res[:, half_s:], in_=rt[:, half_s:]).then_inc(s_out, 16)
            scalar.wait_ge(s_out, 32)

        # ------------------------------------------------ Pool: gathers
        @block.gpsimd
        def _(gpsimd):
            # the Pool engine is idle until the first bucket gate (~40us): it
            # loads the consts, every other early embT chunk, and its own idx
            # tables, so SP's chunk stream (which feeds the PE) never blocks.
            gpsimd.dma_start(out=idxt[0][:], in_=idxa[:]).then_inc(s_cli, 16)
            gpsimd.dma_start(out=idxt[1][:], in_=idxb[:]).then_inc(s_cli, 16)
            for i, cq in enumerate(POOL_CHUNKS):
                if i:
                    gpsimd.wait_ge(s_loadp, 32 * i)  # order s_loadp updates
                load_chunk(gpsimd, cq)
            from concourse import library_config
            gpsimd.load_library(library_config.mlp)
            sizes = ({c[1] for c in calls} | {c[2] for c in calls}) - {0}
            regs = {n: gpsimd.to_reg(n) for n in sorted(sizes)}
            gpsimd.wait_ge(s_cl, CONSTS)
            gpsimd.wait_ge(s_cli, 32)

            def pool_stage0(p):
                bk, n0, n1, icol, scol = calls[p]
                S = (n0 + n1) // 128
                gpsimd.wait_ge(s_ga[p % NBUF], gwait[p])
                gpsimd.wait_ge(s_gb[p % NBUF], gwait[p])
                a3 = at[p % NBUF][:, :S * 128]
                gpsimd.tensor_mul(out=a3, in0=a3,
                                  in1=btg[p % NBUF][:, :S * 128]).then_inc(s_stp, 1)

            # first ep0 pairs of bucket 0 reference only p1 rows < HROWS (the
            # host orders bucket-0 edges early-rows-first), so they unblock
            # after 25 + HROWS/512 projected groups instead of 50.
            gate_req = []
            for p, (bk, n0, n1, icol, scol) in enumerate(calls):
                if bk == 0 and p < ep0:
                    gate_req.append(GPB + HROWS // 512)
                else:
                    gate_req.append(BK_READY_Q[bk])

            cur_gate = -1
            for p, (bk, n0, n1, icol, scol) in enumerate(calls):
                fi, ti = bk // NTB, bk % NTB
                if gate_req[p] > cur_gate:
                    cur_gate = gate_req[p]
                    for r in range(4):
                        gpsimd.wait_ge(s_pw[r], 16 * len(range(r, cur_gate, 4)))
                if p >= NBUF:
                    gpsimd.wait_ge(s_red, p - NBUF + 1)  # at/btg[p%NBUF] consumed
                if p >= 2 and pool_mul[p - 2]:
                    pool_stage0(p - 2)
                rows1 = HROWS if (bk == 0 and p < ep0) else NBP
                # rows are moved as 64 packed f32 (= 128 bf16): same bytes and
                # descriptors, half the modeled element count.
                nh = 0
                for h, n in enumerate((n0, n1)):
                    if n == 0:
                        continue
                    S = n // 128
                    c0 = icol + nh // 16
                    gpsimd.dma_gather(
                        out_ap=at[p % NBUF][:, nh:nh + S * 128].bitcast(F32)
                            .rearrange("p (s d) -> p s d", d=64),
                        in_ap=p1d[fi * NBP:fi * NBP + rows1, :].bitcast(F32),
                        idxs_ap=idxt[0][:, c0:c0 + n // 16],
                        num_idxs=n, num_idxs_reg=regs[n], elem_size=D_OUT // 2,
                        queue_num=0,
                    ).then_inc(s_ga[p % NBUF], 16)
                    gpsimd.dma_gather(
                        out_ap=btg[p % NBUF][:, nh:nh + S * 128].bitcast(F32)
                            .rearrange("p (s d) -> p s d", d=64),
                        in_ap=p2d[ti * NBP:(ti + 1) * NBP, :].bitcast(F32),
                        idxs_ap=idxt[1][:, c0:c0 + n // 16],
                        num_idxs=n, num_idxs_reg=regs[n], elem_size=D_OUT // 2,
                        queue_num=1,
                    ).then_inc(s_gb[p % NBUF], 16)
                    nh += S * 128
            for p in range(max(0, ncall - 2), ncall):
                if pool_mul[p]:
                    pool_stage0(p)

        # ------------------------------------------------ DVE: dot products
        # bf16 multiply (2x mode), tree-halve in bf16 (2x) down to 16 partials
        # per slot, then one short f32 reduce - one chain per gather pair
        # (up to 2048 edges).  The 5-stage chain is software-pipelined across
        # pairs (stage s of pair p in round p+s) so every intra-chain
        # semaphore is already visible when its wait issues.
        @block.vector
        def _(vector):
            # phase-1 assist: odd-group psum->pv copies while gathers are gated
            for q in range(NGROUP):
                if not dve_copy[q]:
                    continue
                vector.wait_ge(s_mm, 12 * q + 12)
                if q >= 4:
                    vector.wait_ge(s_pw[q % 4], 16 * (q // 4))  # pv drained
                vector.tensor_copy(out=pv[q % 4][:], in_=ps[q % 4][:]).then_inc(s_cpd, 1)

            def stage(s, p):
                bk, n0, n1, icol, scol = calls[p]
                S = (n0 + n1) // 128
                av = at[p % NBUF][:, :S * 128].rearrange("p (s d) -> p s d", d=128)
                if s == 0:
                    if pool_mul[p]:
                        return  # multiply issued by the Pool engine
                    vector.wait_ge(s_ga[p % NBUF], gwait[p])
                    vector.wait_ge(s_gb[p % NBUF], gwait[p])
                    a3 = at[p % NBUF][:, :S * 128]
                    vector.tensor_mul(out=a3, in0=a3,
                                      in1=btg[p % NBUF][:, :S * 128]).then_inc(s_st[0], 1)
                elif s == 1:
                    if pool_mul[p]:
                        vector.wait_ge(s_stp, mul_cnt[p][1])
                    else:
                        vector.wait_ge(s_st[0], mul_cnt[p][0])
                    vector.tensor_add(out=av[:, :, 0:64], in0=av[:, :, 0:64],
                                      in1=av[:, :, 64:128]).then_inc(s_st[1], 1)
                elif s in (2, 3):
                    h = 128 >> s  # 32, 16
                    vector.wait_ge(s_st[s - 1], p + 1)
                    vector.tensor_add(out=av[:, :, 0:h], in0=av[:, :, 0:h],
                                      in1=av[:, :, h:2 * h]).then_inc(s_st[s], 1)
                else:
                    vector.wait_ge(s_st[3], p + 1)
                    vector.reduce_sum(out=rt[:, scol:scol + S],
                                      in_=av[:, :, 0:16], axis=AX.X).then_inc(s_red, 1)

            for r in range(ncall + 4):
                for s in range(4, -1, -1):
                    p = r - s
                    if 0 <= p < ncall:
                        stage(s, p)

    return nc, calls, boff


_NC_CACHE: dict = {}


def _get_nc(caps):
    caps, ep0 = caps
    key = (tuple(caps), ep0)
    if key not in _NC_CACHE:
        nc, calls, boff = build_bass(caps, ep0)
        from concourse.library_overlay import lower_extended_insts
        lower_extended_insts(nc)
        _NC_CACHE[key] = (nc, calls, boff)
    return _NC_CACHE[key]


# ---------------------------------------------------------------- host side
def _perm_local(n):
    """block-local node id -> permuted table row (within block).
    Table row g*512 + p*4 + t holds node g*512 + t*128 + p."""
    g, u = np.divmod(n, 512)
    t, p = np.divmod(u, 128)
    return g * 512 + p * 4 + t


def _marshal(emb_1, emb_2, nodes_from_to, W1, b1, W2, b2):
    """Shard/bucket inputs per core.  Returns (caps, in_maps, books)."""
    import ml_dtypes
    bf16 = ml_dtypes.bfloat16

    f = np.asarray(nodes_from_to[:, 0], dtype=np.int64)
    t = np.asarray(nodes_from_to[:, 1], dtype=np.int64)
    emb_1 = np.asarray(emb_1, dtype=np.float32)
    emb_2 = np.asarray(emb_2, dtype=np.float32)
    w12 = np.concatenate(
        [np.asarray(W1, dtype=np.float32), np.asarray(W2, dtype=np.float32)],
        axis=1).astype(bf16)
    bo3 = np.stack([
        np.asarray(b1, dtype=np.float32).reshape(-1),
        np.asarray(b2, dtype=np.float32).reshape(-1),
        np.ones(D_OUT, np.float32),
    ]).astype(bf16)

    core = (f // (NFB * NB)) * 4 + t // (NTB * NB)
    order0 = np.argsort(core, kind="stable")
    ccnt = np.bincount(core, minlength=N_CORES)
    coff = np.concatenate([[0], np.cumsum(ccnt)])

    percore = []
    early_cnts = []
    all_cnts = np.zeros((N_CORES, NBUCKET), np.int64)
    for c in range(N_CORES):
        a, b = c // 4, c % 4
        sel = order0[coff[c]:coff[c + 1]]
        fc, tcv = f[sel], t[sel]
        fi = fc // NB - NFB * a
        ti = tcv // NB - NTB * b
        fl = _perm_local(fc % NB).astype(np.int16)
        tl = _perm_local(tcv % NB).astype(np.int16)
        bk = fi * NTB + ti
        o2 = np.argsort(bk, kind="stable")
        sel2, fl2, tl2 = sel[o2], fl[o2], tl[o2]
        cnts = np.bincount(bk, minlength=NBUCKET)
        all_cnts[c] = cnts
        # bucket 0: early (prefix-row) edges first, enabling the half-gate
        n0 = cnts[0]
        e0 = fl2[:n0] < HROWS
        o3 = np.argsort(~e0, kind="stable")
        sel2[:n0], fl2[:n0], tl2[:n0] = sel2[:n0][o3], fl2[:n0][o3], tl2[:n0][o3]
        early_cnts.append(int(e0.sum()))
        percore.append((a, b, sel2, fl2, tl2, cnts))

    caps = [int(-(-all_cnts[:, k].max() // 128) * 128) for k in range(NBUCKET)]
    ep0 = min(early_cnts) // (2 * MAX_CALL)
    calls, idx_cols, tot_slots, boff = _plan_calls(caps)

    in_maps, books = [], []
    for c in range(N_CORES):
        a, b, sel2, fl2, tl2, cnts = percore[c]
        pos = np.concatenate([[0], np.cumsum(cnts)])

        slots_a = np.zeros((NBUCKET, max(caps)), np.int16)
        slots_b = np.zeros((NBUCKET, max(caps)), np.int16)
        for k in range(NBUCKET):
            slots_a[k, :cnts[k]] = fl2[pos[k]:pos[k + 1]]
            slots_b[k, :cnts[k]] = tl2[pos[k]:pos[k + 1]]
        # wrap by 16: idx i of a bucket at (partition i%16, col i//16),
        # replicated across the 8 groups of 16 partitions
        wa_cols = []
        wb_cols = []
        for k in range(NBUCKET):
            cap = caps[k]
            wa_cols.append(slots_a[k, :cap].reshape(cap // 16, 16).T)
            wb_cols.append(slots_b[k, :cap].reshape(cap // 16, 16).T)
        idxa = np.tile(np.concatenate(wa_cols, axis=1), (8, 1))
        idxb = np.tile(np.concatenate(wb_cols, axis=1), (8, 1))

        e1t = np.zeros((D_IN, P1_ROWS), bf16)
        for i in range(NFB):
            blk = emb_1[(NFB * a + i) * NB:(NFB * a + i + 1) * NB]
            e1t[:, i * NBP:i * NBP + NB] = blk.T.astype(bf16)
        e2t = np.zeros((D_IN, P2_ROWS), bf16)
        for i in range(NTB):
            blk = emb_2[(NTB * b + i) * NB:(NTB * b + i + 1) * NB]
            e2t[:, i * NBP:i * NBP + NB] = blk.T.astype(bf16)

        in_maps.append({
            "e1t": e1t, "e2t": e2t, "w12": w12, "bo3": bo3,
            "idxa": np.ascontiguousarray(idxa),
            "idxb": np.ascontiguousarray(idxb),
        })
        books.append((sel2, cnts, pos))
    return (caps, ep0), in_maps, books


def _unmarshal(results, books, caps, n_edges):
    calls, idx_cols, tot_slots, boff = _plan_calls(caps[0])
    out = np.empty(n_edges, np.float32)
    for c in range(N_CORES):
        sel2, cnts, pos = books[c]
        r = results[c]["res"]  # [128, tot_slots]
        for k in range(NBUCKET):
            if cnts[k] == 0:
                continue
            s0 = boff[k]
            nslots = caps[0][k] // 128
            stream = r[:, s0:s0 + nslots].T.reshape(-1)
            out[sel2[pos[k]:pos[k + 1]]] = stream[:cnts[k]]
    return out


def _run(inputs, trace=False, **run_kwargs):
    from concourse.bass_utils import run_bass_kernel_spmd

    caps, in_maps, books = _marshal(**inputs)
    nc, calls, boff = _get_nc(caps)
    r = run_bass_kernel_spmd(
        nc, in_maps, core_ids=list(range(N_CORES)), trace=trace, **run_kwargs
    )
    out = _unmarshal(r.results, books, caps, len(inputs["nodes_from_to"]))
    return out, r


def kernel(**inputs) -> np.ndarray:
    out, _ = _run(inputs, trace=False)
    return out


# revision 77
# speedup vs baseline: 2.4151x; 1.0081x over previous
"""Trainium2 Bass kernel for nn_BetweenClusterFC.

Computes out[e] = (emb_1[f[e]] @ W1 + b1) . (emb_2[t[e]] @ W2 + b2)
for E = 1.6M edges over N = 100k nodes, D_IN = 256, D_OUT = 128.

Strategy (8 NeuronCores, SPMD, full inputs in / full output out):
  - Nodes are split into 8 blocks of 12500.  Edges are assigned to cores by a
    (from-block-group, to-block-group) 4x2 rectangle: core c=(a,b) handles
    edges with from-node in blocks [4a..4a+3] and to-node in blocks
    [2b..2b+1].  Uniform (~200k edges/core); each core needs projections for
    4 from-blocks + 2 to-blocks (75k nodes).
  - All streamed data is bf16: the host pre-transposes and casts the
    embedding shards; the PE projects p = emb @ W (+ bias folded in as a
    K=1 ones x bias matmul), the Act engine moves psum -> sbuf in bf16 and
    issues the p-table writes (1KB-contiguous via a (p t)-permuted row
    order), keeping the DVE entirely free for the edge phase.
  - Edges are bucketed host-side by (local from-block, local to-block); per
    bucket both endpoint rows are fetched with SWDGE dma_gather calls (1024
    idxs/call HW limit, paired so the DVE sees 2048-edge slabs, two DMA
    queues, rows moved as packed f32 pairs) using int16 permuted-local
    indices; the DVE runs a software-pipelined bf16 multiply (2x mode) +
    bf16 tree-halving + short f32 reduce per slab into a single
    SBUF-resident result tile, written out in two halves.
  - Bucket capacities are sized from the actual per-core edge counts (max
    across cores, 128-aligned), so the Bass program is built per problem
    instance (inspector-executor style) and cached.  Bucket 0 is ordered
    early-prefix-rows first so its first gather pairs unblock after 38 of
    150 projection groups; the idle pre-gather Pool engine prefetches the
    idx tables and part of the embT stream; the DVE covers half the early
    psum->sbuf copies; p-table writes go out as two-group pair DMAs to
    amortize the per-DMA descriptor-generation floor.
  - The host applies the inverse edge permutation to assemble the output.

Written in raw Bass (explicit semaphores) - the Tile layer's generated sync
exceeds this toolchain's per-instruction wait-slot limits.
"""

import contextlib

import numpy as np

import concourse.bass as bass
import concourse.mybir as mybir

# ---------------------------------------------------------------- constants
N_NODES = 100_000
D_IN = 256
D_OUT = 128
N_EDGES = 1_600_000
N_CORES = 8

NB = 12_500          # nodes per block
NBP = 12_800         # padded block rows (25 * 512) - multiple of 512
NFB = 4              # from-blocks per core
NTB = 2              # to-blocks per core
NBUCKET = NFB * NTB  # 8 buckets per core

P1_ROWS = NFB * NBP  # 51200
P2_ROWS = NTB * NBP  # 25600

TILES1 = P1_ROWS // 128    # 400 node-tiles, table 1
TILES2 = P2_ROWS // 128    # 200 node-tiles, table 2
GROUPS1 = TILES1 // 4      # 100 psum groups
GROUPS2 = TILES2 // 4      # 50
NGROUP = GROUPS1 + GROUPS2  # 150
CHUNK_T = 20               # node-tiles per embT load chunk (= 5 groups)
NCH1 = TILES1 // CHUNK_T   # 20 chunks
NCH2 = TILES2 // CHUNK_T   # 10
NCHUNK = NCH1 + NCH2       # 30
EMB_COLS = CHUNK_T * 128   # 2560

MAX_CALL = 1024            # dma_gather idxs per call (HW limit; ring 4096/queue)
HROWS = 6_656              # p1-block-0 prefix rows (13 groups; pair-even gate)
PH1 = 48                   # DVE assists psum->pv copies for odd groups < PH1

F32 = mybir.dt.float32
BF16 = mybir.dt.bfloat16
I16 = mybir.dt.int16
AX = mybir.AxisListType


# Projection block order: interleave p2/p1 blocks so gather buckets unblock
# as early as possible (bucket (fi,ti) needs p1 block fi + p2 block ti):
# (0,0)@50 groups, (0,1)@75, (1,*)@100, (2,*)@125, (3,*)@150.
BSEQ = [(1, 0), (0, 0), (1, 1), (0, 1), (0, 2), (0, 3)]
GPB = NBP // 512   # 25 groups per block
CPB = GPB // 5     # 5 chunks per block (CHUNK_T = 4 groups... 20 tiles)
GSEQ = []
CSEQ = []
for tab, blk in BSEQ:
    g0 = blk * GPB + (GROUPS1 if tab == 1 else 0)
    GSEQ += list(range(g0, g0 + GPB))
    c0 = blk * 5 + (NCH1 if tab == 1 else 0)
    CSEQ += list(range(c0, c0 + 5))
CPOS = {cid: q for q, cid in enumerate(CSEQ)}
_BPOS = {tb: i for i, tb in enumerate(BSEQ)}
# bucket (fi,ti) ready after this many groups in GSEQ order
BK_READY_Q = [GPB * (1 + max(_BPOS[(0, fi)], _BPOS[(1, ti)]))
              for fi in range(NFB) for ti in range(NTB)]


def _chunk_of_tile(tg):
    """global tile index -> (global chunk id, local col0 within chunk)."""
    if tg < TILES1:
        c = tg // CHUNK_T
        return c, (tg % CHUNK_T) * 128
    t2 = tg - TILES1
    c = NCH1 + t2 // CHUNK_T
    return c, (t2 % CHUNK_T) * 128


def _chunk_src(c):
    """global chunk id -> (table, col0)."""
    if c < NCH1:
        return 0, c * EMB_COLS
    return 1, (c - NCH1) * EMB_COLS


def _plan_calls(caps):
    """caps: per-bucket slot capacities (multiples of 128).
    Returns a list of gather-call PAIRS (bucket, n0, n1, idx_col0, slot_col0):
    two <=1024-idx gather calls whose outputs land adjacently so the DVE can
    process 2048 edges per op chain (n1 == 0 for a lone remainder call).
    Plus (idx_cols_total, slot_cols_total, per-bucket slot offsets)."""
    pairs = []
    icol = 0
    scol = 0
    boff = []
    for bk, cap in enumerate(caps):
        boff.append(scol)
        left = cap
        while left > 0:
            n0 = min(MAX_CALL, left)
            n1 = min(MAX_CALL, left - n0)
            pairs.append((bk, n0, n1, icol, scol))
            icol += (n0 + n1) // 16
            scol += (n0 + n1) // 128
            left -= n0 + n1
    return pairs, icol, scol, boff


# ---------------------------------------------------------------- device code
def build_bass(caps, ep0):
    """caps: per-bucket capacities (edge slots, multiples of 128), shared by
    all cores (max over cores).  ep0: bucket-0 pairs gathering only the
    HROWS-row prefix of p1 block 0 (host orders those edges first)."""
    calls, idx_cols, tot_slots, boff = _plan_calls(caps)
    ncall = len(calls)

    nc = bass.Bass(dynamic_dma_scratch_size=65536, num_swdge_queues=2)

    e1t = nc.dram_tensor("e1t", [D_IN, P1_ROWS], BF16, kind="ExternalInput")
    e2t = nc.dram_tensor("e2t", [D_IN, P2_ROWS], BF16, kind="ExternalInput")
    w12 = nc.dram_tensor("w12", [D_IN, 2 * D_OUT], BF16, kind="ExternalInput")
    bo3 = nc.dram_tensor("bo3", [3, D_OUT], BF16, kind="ExternalInput")
    idxa = nc.dram_tensor("idxa", [128, idx_cols], I16, kind="ExternalInput")
    idxb = nc.dram_tensor("idxb", [128, idx_cols], I16, kind="ExternalInput")
    res = nc.dram_tensor("res", [128, tot_slots], F32, kind="ExternalOutput")

    p1d = nc.dram_tensor("p1d", [P1_ROWS, D_OUT], BF16, kind="Internal")
    p2d = nc.dram_tensor("p2d", [P2_ROWS, D_OUT], BF16, kind="Internal")
    pdst = (p1d, p2d)

    st = contextlib.ExitStack()
    with st:
        sb = lambda nm, shape, dt: st.enter_context(nc.sbuf_tensor(nm, shape, dt))
        sem = lambda nm: st.enter_context(nc.semaphore(name=nm))

        # wc columns: [w1_k0 | w2_k0 | w1_k1 | w2_k1] (128 each)
        wc = sb("wc", [128, 512], BF16)
        bt = [sb(f"bt{i}", [1, 128], BF16) for i in range(3)]  # b1, b2, ones
        idxt = (sb("idxta", [128, idx_cols], I16), sb("idxtb", [128, idx_cols], I16))
        et = [[sb(f"et_{p}_{k}", [128, EMB_COLS], BF16) for k in range(2)]
              for p in range(3)]  # [buffer cq%3][k-half]
        pvt = sb("pvt", [128, 4 * 512], BF16)
        pv = [pvt[:, i * 512:(i + 1) * 512] for i in range(4)]
        ps = [st.enter_context(nc.psum_tensor(f"ps{i}", [128, 512], F32))
              for i in range(4)]
        NBUF = 6  # gather-pair buffer rotation depth (5-stage DVE pipeline)
        at = [sb(f"at{i}", [128, 2 * MAX_CALL], BF16) for i in range(NBUF)]
        btg = [sb(f"btg{i}", [128, 2 * MAX_CALL], BF16) for i in range(NBUF)]
        rt = sb("rt", [128, tot_slots], F32)

        s_cl = sem("s_cl")               # w/b const loads (5 dmas -> 80)
        s_cli = sem("s_cli")             # idx table loads (2 dmas -> 32)
        s_load = tuple(sem(f"s_load{i}") for i in range(3))  # SP embT, by cq%3
        s_loadp = sem("s_loadp")         # Pool embT loads
        s_mm = sem("s_mm")               # matmuls (+1 each; 3 per tile)
        s_cp = sem("s_cp")               # act copies (+1 each)
        s_cpd = sem("s_cpd")             # dve copies (+1 each)
        s_pw = tuple(sem(f"s_pw{i}") for i in range(2))  # pair writes, by j%2
        s_pwx = sem("s_pwx")             # first halves of crossing pairs
        # gather-completion sems by k%NBUF per queue: call k+NBUF's gathers are
        # gated on s_red >= k+1, so updates of each sem are causally ordered.
        s_ga = tuple(sem(f"s_ga{i}") for i in range(6))  # queue-0, +16/call
        s_gb = tuple(sem(f"s_gb{i}") for i in range(6))  # queue-1, +16/call
        s_st = tuple(sem(f"s_st{i}") for i in range(4))  # DVE dot stages 0-3
        s_stp = sem("s_stp")             # Pool-issued muls (stage 0)
        s_red = sem("s_red")             # final reduces (+1 per call)
        s_out = sem("s_out")             # res dma

        CONSTS = 5 * 16

        # per-sem cumulative gather counts: pair p incs s_g*[p%NBUF] by
        # 16 per issued call (2 for a full pair, 1 for a lone remainder)
        _sem_cnt = [0] * NBUF
        gwait = []
        for _p, (_bk, _n0, _n1, _i, _s) in enumerate(calls):
            _sem_cnt[_p % NBUF] += 16 * (2 if _n1 else 1)
            gwait.append(_sem_cnt[_p % NBUF])

        # pairs whose stage-0 multiply runs on the Pool engine.  Disabled: the
        # Pool must keep the mlp ucode library loaded for dma_gather, which
        # excludes the standard library's TensorTensor.
        pool_mul = [False] * ncall

        # During the pre-gather phase the DVE is mostly idle: it takes the odd
        # groups' psum->pv copies for the first PH1 groups.
        dve_copy = [q < PH1 and q % 2 == 1 for q in range(NGROUP)]
        cp_cnt = []  # cumulative (act, dve) copy counts through group q
        _na = _ndv = 0
        for q in range(NGROUP):
            if dve_copy[q]:
                _ndv += 1
            else:
                _na += 1
            cp_cnt.append((_na, _ndv))

        def _copy_wait(eng, q):
            """wait until the copy of group q is complete."""
            if dve_copy[q]:
                eng.wait_ge(s_cpd, cp_cnt[q][1])
            else:
                eng.wait_ge(s_cp, cp_cnt[q][0])
        mul_cnt = []
        _nd = _np_ = 0
        for p in range(ncall):
            if pool_mul[p]:
                _np_ += 1
            else:
                _nd += 1
            mul_cnt.append((_nd, _np_))

        block = st.enter_context(nc.Block())

        # Chunks 1,3,5,7,9 are loaded by the Pool engine during its pre-gather
        # idle window (own sem s_loadp); SP loads the rest (s_load by parity).
        POOL_CHUNKS = (4, 6, 8)
        chunk_wait = {}
        _cnt = {0: 0, 1: 0, 2: 0, 'p': 0}
        for _cq in range(NCHUNK):
            key = 'p' if _cq in POOL_CHUNKS else _cq % 3
            _cnt[key] += 32
            chunk_wait[_cq] = (key, _cnt[key])

        # p-writes happen per PAIR of groups (2j, 2j+1): pv slots are adjacent
        # so one DMA covers both, halving Act's per-write descriptor-gen
        # floor.  Pairs whose two groups land in different projection blocks
        # ("crossing": non-contiguous table rows) are written as two singles,
        # the first tracked by s_pwx.
        NPAIR_W = NGROUP // 2
        CROSSING = frozenset(j for j in range(NPAIR_W)
                             if GSEQ[2 * j + 1] != GSEQ[2 * j] + 1)
        _xrank = {j: i + 1 for i, j in enumerate(sorted(CROSSING))}

        def _one_write(eng, q, s_sem, cnt):
            g = GSEQ[q]
            tab = 0 if g < GROUPS1 else 1
            r0 = g * 512 if tab == 0 else (g - GROUPS1) * 512
            # (p t)-permuted write: table row r0 + p*4 + t <- node r0 + t*128 + p
            eng.dma_start(
                out=pdst[tab][r0:r0 + 512, :].rearrange("(p t) d -> p t d", p=128),
                in_=pv[q % 4][:].rearrange("p (t d) -> p t d", d=128),
            ).then_inc(s_sem, cnt)

        def p_write_pair(eng, j):
            if j in CROSSING:
                _one_write(eng, 2 * j, s_pwx, 16)
                _one_write(eng, 2 * j + 1, s_pw[j % 2], 16)
                return
            q0 = 2 * j
            g = GSEQ[q0]
            tab = 0 if g < GROUPS1 else 1
            r0 = g * 512 if tab == 0 else (g - GROUPS1) * 512
            s0 = q0 % 4
            eng.dma_start(
                out=pdst[tab][r0:r0 + 1024, :]
                    .rearrange("(g p t) d -> p g t d", p=128, t=4),
                in_=pvt[:, s0 * 512:(s0 + 2) * 512]
                    .rearrange("p (g t d) -> p g t d", t=4, d=128),
            ).then_inc(s_pw[j % 2], 16)

        def wait_pairs_through(eng, nq):
            """wait until all p-writes for groups < nq are complete (nq is
            rounded up to a pair boundary)."""
            npair = (nq + 1) // 2
            for r in range(2):
                eng.wait_ge(s_pw[r], 16 * len(range(r, npair, 2)))
            nx = sum(1 for j in CROSSING if j < npair)
            if nx:
                eng.wait_ge(s_pwx, 16 * nx)

        def wait_pv_drained(eng, q):
            """wait until pv slot q%4 (last used by group q-4) is rewritable."""
            p4 = (q - 4) // 2
            eng.wait_ge(s_pw[p4 % 2], 16 * (p4 // 2 + 1))
            if p4 in CROSSING:
                eng.wait_ge(s_pwx, 16 * _xrank[p4])

        def load_chunk(eng, cq):
            if cq >= 3:
                # buffer cq%3 previously held chunk cq-3; wait consumed
                eng.wait_ge(s_mm, 3 * CHUNK_T * (cq - 2))
            tab, col0 = _chunk_src(CSEQ[cq])
            src = e1t if tab == 0 else e2t
            par = cq % 3
            s_ld = s_loadp if cq in POOL_CHUNKS else s_load[cq % 3]
            eng.dma_start(out=et[par][0][:],
                          in_=src[0:128, col0:col0 + EMB_COLS]).then_inc(s_ld, 16)
            eng.dma_start(out=et[par][1][:],
                          in_=src[128:256, col0:col0 + EMB_COLS]).then_inc(s_ld, 16)

        # ------------------------------------------------ SP: embT + p-writes
        @block.sync
        def _(sync):
            for k in range(2):
                sync.dma_start(out=wc[:, k * 256:(k + 1) * 256],
                               in_=w12[k * 128:(k + 1) * 128, :]).then_inc(s_cl, 16)
            for i in range(3):
                sync.dma_start(out=bt[i][:], in_=bo3[i:i + 1, :]).then_inc(s_cl, 16)
            load_chunk(sync, 0)
            load_chunk(sync, 1)
            next_cq = 2
            for q in range(NGROUP):
                # look ahead: issue loads for chunks starting within 5 groups
                while next_cq < NCHUNK and next_cq * CHUNK_T <= (q + 5) * 4 + 3:
                    if next_cq not in POOL_CHUNKS:
                        load_chunk(sync, next_cq)
                    next_cq += 1


        # ------------------------------------------------ PE: projections
        @block.tensor
        def _(tensor):
            tensor.wait_ge(s_cl, CONSTS)
            for q, g in enumerate(GSEQ):
                tab = 0 if g < GROUPS1 else 1
                bc = bt[tab][:]
                for j in range(4):
                    tq = q * 4 + j
                    cid, col0 = _chunk_of_tile(g * 4 + j)
                    cq = CPOS[cid]
                    if tq == cq * CHUNK_T:  # first processed tile of chunk
                        key, cntv = chunk_wait[cq]
                        tensor.wait_ge(s_loadp if key == 'p' else s_load[key], cntv)
                    if j == 0 and q >= 4:
                        _copy_wait(tensor, q - 4)  # psum bank q%4 free
                    out = ps[q % 4][:, j * 128:(j + 1) * 128]
                    tensor.matmul(out=out, lhsT=et[cq % 3][0][:, col0:col0 + 128],
                                  rhs=wc[:, tab * 128:tab * 128 + 128],
                                  start=True, stop=False).then_inc(s_mm, 1)
                    tensor.matmul(out=out, lhsT=et[cq % 3][1][:, col0:col0 + 128],
                                  rhs=wc[:, 256 + tab * 128:256 + tab * 128 + 128],
                                  start=False, stop=False).then_inc(s_mm, 1)
                    tensor.matmul(out=out, lhsT=bt[2][:], rhs=bc,
                                  start=False, stop=True).then_inc(s_mm, 1)

        # ------------------------------------------------ Act: psum->sbuf + DMAs
        @block.scalar
        def _(scalar):
            scalar.wait_ge(s_cl, CONSTS)
            # the pair write of groups (q-2, q-1) is issued after the copy of
            # group q so the copy-completion sems are already visible.
            for q, g in enumerate(GSEQ):
                if not dve_copy[q]:
                    scalar.wait_ge(s_mm, 12 * q + 12)
                    if q >= 4:
                        wait_pv_drained(scalar, q)
                    scalar.copy(out=pv[q % 4][:], in_=ps[q % 4][:]).then_inc(s_cp, 1)
                if q >= 2 and q % 2 == 0:
                    _copy_wait(scalar, q - 2)
                    _copy_wait(scalar, q - 1)
                    p_write_pair(scalar, (q - 2) // 2)
            _copy_wait(scalar, NGROUP - 2)
            _copy_wait(scalar, NGROUP - 1)
            p_write_pair(scalar, NPAIR_W - 1)
            # write results in two halves so only the last sliver is exposed
            half_p = ncall // 2
            half_s = calls[half_p][4]
            scalar.wait_ge(s_red, half_p)
            scalar.dma_start(out=res[:, :half_s], in_=rt[:, :half_s]).then_inc(s_out, 16)
            scalar.wait_ge(s_red, ncall)
            scalar.dma_start(out=res[:, half_s:], in_=rt[:, half_s:]).then_inc(s_out, 16)
            scalar.wait_ge(s_out, 32)

        # ------------------------------------------------ Pool: gathers
        @block.gpsimd
        def _(gpsimd):
            # the Pool engine is idle until the first bucket gate (~40us): it
            # loads the consts, every other early embT chunk, and its own idx
            # tables, so SP's chunk stream (which feeds the PE) never blocks.
            gpsimd.dma_start(out=idxt[0][:], in_=idxa[:]).then_inc(s_cli, 16)
            gpsimd.dma_start(out=idxt[1][:], in_=idxb[:]).then_inc(s_cli, 16)
            for i, cq in enumerate(POOL_CHUNKS):
                if i:
                    gpsimd.wait_ge(s_loadp, 32 * i)  # order s_loadp updates
                load_chunk(gpsimd, cq)
            from concourse import library_config
            gpsimd.load_library(library_config.mlp)
            sizes = ({c[1] for c in calls} | {c[2] for c in calls}) - {0}
            regs = {n: gpsimd.to_reg(n) for n in sorted(sizes)}
            gpsimd.wait_ge(s_cl, CONSTS)
            gpsimd.wait_ge(s_cli, 32)

            def pool_stage0(p):
                bk, n0, n1, icol, scol = calls[p]
                S = (n0 + n1) // 128
                gpsimd.wait_ge(s_ga[p % NBUF], gwait[p])
                gpsimd.wait_ge(s_gb[p % NBUF], gwait[p])
                a3 = at[p % NBUF][:, :S * 128]
                gpsimd.tensor_mul(out=a3, in0=a3,
                                  in1=btg[p % NBUF][:, :S * 128]).then_inc(s_stp, 1)

            # first ep0 pairs of bucket 0 reference only p1 rows < HROWS (the
            # host orders bucket-0 edges early-rows-first), so they unblock
            # after 25 + HROWS/512 projected groups instead of 50.
            gate_req = []
            for p, (bk, n0, n1, icol, scol) in enumerate(calls):
                if bk == 0 and p < ep0:
                    gate_req.append(GPB + HROWS // 512)
                else:
                    gate_req.append(BK_READY_Q[bk])

            cur_gate = -1
            for p, (bk, n0, n1, icol, scol) in enumerate(calls):
                fi, ti = bk // NTB, bk % NTB
                if gate_req[p] > cur_gate:
                    cur_gate = gate_req[p]
                    wait_pairs_through(gpsimd, cur_gate)
                if p >= NBUF:
                    gpsimd.wait_ge(s_red, p - NBUF + 1)  # at/btg[p%NBUF] consumed
                if p >= 2 and pool_mul[p - 2]:
                    pool_stage0(p - 2)
                rows1 = HROWS if (bk == 0 and p < ep0) else NBP
                # rows are moved as 64 packed f32 (= 128 bf16): same bytes and
                # descriptors, half the modeled element count.
                nh = 0
                for h, n in enumerate((n0, n1)):
                    if n == 0:
                        continue
                    S = n // 128
                    c0 = icol + nh // 16
                    gpsimd.dma_gather(
                        out_ap=at[p % NBUF][:, nh:nh + S * 128].bitcast(F32)
                            .rearrange("p (s d) -> p s d", d=64),
                        in_ap=p1d[fi * NBP:fi * NBP + rows1, :].bitcast(F32),
                        idxs_ap=idxt[0][:, c0:c0 + n // 16],
                        num_idxs=n, num_idxs_reg=regs[n], elem_size=D_OUT // 2,
                        queue_num=0,
                    ).then_inc(s_ga[p % NBUF], 16)
                    gpsimd.dma_gather(
                        out_ap=btg[p % NBUF][:, nh:nh + S * 128].bitcast(F32)
                            .rearrange("p (s d) -> p s d", d=64),
                        in_ap=p2d[ti * NBP:(ti + 1) * NBP, :].bitcast(F32),
                        idxs_ap=idxt[1][:, c0:c0 + n // 16],
                        num_idxs=n, num_idxs_reg=regs[n], elem_size=D_OUT // 2,
                        queue_num=1,
                    ).then_inc(s_gb[p % NBUF], 16)
                    nh += S * 128
            for p in range(max(0, ncall - 2), ncall):
                if pool_mul[p]:
                    pool_stage0(p)

        # ------------------------------------------------ DVE: dot products
        # bf16 multiply (2x mode), tree-halve in bf16 (2x) down to 16 partials
        # per slot, then one short f32 reduce - one chain per gather pair
        # (up to 2048 edges).  The 5-stage chain is software-pipelined across
        # pairs (stage s of pair p in round p+s) so every intra-chain
        # semaphore is already visible when its wait issues.
        @block.vector
        def _(vector):
            # phase-1 assist: odd-group psum->pv copies while gathers are gated
            for q in range(NGROUP):
                if not dve_copy[q]:
                    continue
                vector.wait_ge(s_mm, 12 * q + 12)
                if q >= 4:
                    wait_pv_drained(vector, q)
                vector.tensor_copy(out=pv[q % 4][:], in_=ps[q % 4][:]).then_inc(s_cpd, 1)

            def stage(s, p):
                bk, n0, n1, icol, scol = calls[p]
                S = (n0 + n1) // 128
                av = at[p % NBUF][:, :S * 128].rearrange("p (s d) -> p s d", d=128)
                if s == 0:
                    if pool_mul[p]:
                        return  # multiply issued by the Pool engine
                    vector.wait_ge(s_ga[p % NBUF], gwait[p])
                    vector.wait_ge(s_gb[p % NBUF], gwait[p])
                    a3 = at[p % NBUF][:, :S * 128]
                    vector.tensor_mul(out=a3, in0=a3,
                                      in1=btg[p % NBUF][:, :S * 128]).then_inc(s_st[0], 1)
                elif s == 1:
                    if pool_mul[p]:
                        vector.wait_ge(s_stp, mul_cnt[p][1])
                    else:
                        vector.wait_ge(s_st[0], mul_cnt[p][0])
                    vector.tensor_add(out=av[:, :, 0:64], in0=av[:, :, 0:64],
                                      in1=av[:, :, 64:128]).then_inc(s_st[1], 1)
                elif s in (2, 3):
                    h = 128 >> s  # 32, 16
                    vector.wait_ge(s_st[s - 1], p + 1)
                    vector.tensor_add(out=av[:, :, 0:h], in0=av[:, :, 0:h],
                                      in1=av[:, :, h:2 * h]).then_inc(s_st[s], 1)
                else:
                    vector.wait_ge(s_st[3], p + 1)
                    vector.reduce_sum(out=rt[:, scol:scol + S],
                                      in_=av[:, :, 0:16], axis=AX.X).then_inc(s_red, 1)

            for r in range(ncall + 4):
                for s in range(4, -1, -1):
                    p = r - s
                    if 0 <= p < ncall:
                        stage(s, p)

    return nc, calls, boff


_NC_CACHE: dict = {}


def _get_nc(caps):
    caps, ep0 = caps
    key = (tuple(caps), ep0)
    if key not in _NC_CACHE:
        nc, calls, boff = build_bass(caps, ep0)
        from concourse.library_overlay import lower_extended_insts
        lower_extended_insts(nc)
        _NC_CACHE[key] = (nc, calls, boff)
    return _NC_CACHE[key]


# ---------------------------------------------------------------- host side
def _perm_local(n):
    """block-local node id -> permuted table row (within block).
    Table row g*512 + p*4 + t holds node g*512 + t*128 + p."""
    g, u = np.divmod(n, 512)
    t, p = np.divmod(u, 128)
    return g * 512 + p * 4 + t


def _marshal(emb_1, emb_2, nodes_from_to, W1, b1, W2, b2):
    """Shard/bucket inputs per core.  Returns (caps, in_maps, books)."""
    import ml_dtypes
    bf16 = ml_dtypes.bfloat16

    f = np.asarray(nodes_from_to[:, 0], dtype=np.int64)
    t = np.asarray(nodes_from_to[:, 1], dtype=np.int64)
    emb_1 = np.asarray(emb_1, dtype=np.float32)
    emb_2 = np.asarray(emb_2, dtype=np.float32)
    w12 = np.concatenate(
        [np.asarray(W1, dtype=np.float32), np.asarray(W2, dtype=np.float32)],
        axis=1).astype(bf16)
    bo3 = np.stack([
        np.asarray(b1, dtype=np.float32).reshape(-1),
        np.asarray(b2, dtype=np.float32).reshape(-1),
        np.ones(D_OUT, np.float32),
    ]).astype(bf16)

    core = (f // (NFB * NB)) * 4 + t // (NTB * NB)
    order0 = np.argsort(core, kind="stable")
    ccnt = np.bincount(core, minlength=N_CORES)
    coff = np.concatenate([[0], np.cumsum(ccnt)])

    percore = []
    early_cnts = []
    all_cnts = np.zeros((N_CORES, NBUCKET), np.int64)
    for c in range(N_CORES):
        a, b = c // 4, c % 4
        sel = order0[coff[c]:coff[c + 1]]
        fc, tcv = f[sel], t[sel]
        fi = fc // NB - NFB * a
        ti = tcv // NB - NTB * b
        fl = _perm_local(fc % NB).astype(np.int16)
        tl = _perm_local(tcv % NB).astype(np.int16)
        bk = fi * NTB + ti
        o2 = np.argsort(bk, kind="stable")
        sel2, fl2, tl2 = sel[o2], fl[o2], tl[o2]
        cnts = np.bincount(bk, minlength=NBUCKET)
        all_cnts[c] = cnts
        # bucket 0: early (prefix-row) edges first, enabling the half-gate
        n0 = cnts[0]
        e0 = fl2[:n0] < HROWS
        o3 = np.argsort(~e0, kind="stable")
        sel2[:n0], fl2[:n0], tl2[:n0] = sel2[:n0][o3], fl2[:n0][o3], tl2[:n0][o3]
        early_cnts.append(int(e0.sum()))
        percore.append((a, b, sel2, fl2, tl2, cnts))

    caps = [int(-(-all_cnts[:, k].max() // 128) * 128) for k in range(NBUCKET)]
    ep0 = min(early_cnts) // (2 * MAX_CALL)
    calls, idx_cols, tot_slots, boff = _plan_calls(caps)

    in_maps, books = [], []
    for c in range(N_CORES):
        a, b, sel2, fl2, tl2, cnts = percore[c]
        pos = np.concatenate([[0], np.cumsum(cnts)])

        slots_a = np.zeros((NBUCKET, max(caps)), np.int16)
        slots_b = np.zeros((NBUCKET, max(caps)), np.int16)
        for k in range(NBUCKET):
            slots_a[k, :cnts[k]] = fl2[pos[k]:pos[k + 1]]
            slots_b[k, :cnts[k]] = tl2[pos[k]:pos[k + 1]]
        # wrap by 16: idx i of a bucket at (partition i%16, col i//16),
        # replicated across the 8 groups of 16 partitions
        wa_cols = []
        wb_cols = []
        for k in range(NBUCKET):
            cap = caps[k]
            wa_cols.append(slots_a[k, :cap].reshape(cap // 16, 16).T)
            wb_cols.append(slots_b[k, :cap].reshape(cap // 16, 16).T)
        idxa = np.tile(np.concatenate(wa_cols, axis=1), (8, 1))
        idxb = np.tile(np.concatenate(wb_cols, axis=1), (8, 1))

        e1t = np.zeros((D_IN, P1_ROWS), bf16)
        for i in range(NFB):
            blk = emb_1[(NFB * a + i) * NB:(NFB * a + i + 1) * NB]
            e1t[:, i * NBP:i * NBP + NB] = blk.T.astype(bf16)
        e2t = np.zeros((D_IN, P2_ROWS), bf16)
        for i in range(NTB):
            blk = emb_2[(NTB * b + i) * NB:(NTB * b + i + 1) * NB]
            e2t[:, i * NBP:i * NBP + NB] = blk.T.astype(bf16)

        in_maps.append({
            "e1t": e1t, "e2t": e2t, "w12": w12, "bo3": bo3,
            "idxa": np.ascontiguousarray(idxa),
            "idxb": np.ascontiguousarray(idxb),
        })
        books.append((sel2, cnts, pos))
    return (caps, ep0), in_maps, books


def _unmarshal(results, books, caps, n_edges):
    calls, idx_cols, tot_slots, boff = _plan_calls(caps[0])
    out = np.empty(n_edges, np.float32)
    for c in range(N_CORES):
        sel2, cnts, pos = books[c]
        r = results[c]["res"]  # [128, tot_slots]
        for k in range(NBUCKET):
            if cnts[k] == 0:
                continue
            s0 = boff[k]
            nslots = caps[0][k] // 128
            stream = r[:, s0:s0 + nslots].T.reshape(-1)
            out[sel2[pos[k]:pos[k + 1]]] = stream[:cnts[k]]
    return out


def _run(inputs, trace=False, **run_kwargs):
    from concourse.bass_utils import run_bass_kernel_spmd

    caps, in_maps, books = _marshal(**inputs)
    nc, calls, boff = _get_nc(caps)
    r = run_bass_kernel_spmd(
        nc, in_maps, core_ids=list(range(N_CORES)), trace=trace, **run_kwargs
    )
    out = _unmarshal(r.results, books, caps, len(inputs["nodes_from_to"]))
    return out, r


def kernel(**inputs) -> np.ndarray:
    out, _ = _run(inputs, trace=False)
    return out
